# revision 1
# baseline (speedup 1.0000x reference)
"""AdaConv2D Trainium2 Bass kernel.

Problem (per sample): instance-norm(x) -> grouped 3x3 conv (128 groups,
2ch/group, per-sample weights) -> grouped 1x1 conv -> +bias.
B=8, Cin=Cout=256, H=W=128.

Strategy: pure data-parallel, 1 sample per NeuronCore (8 cores).

Per-core algorithm:
  - The 1x1 grouped conv is folded into the 3x3 weights:
        w_eff[co, j, t] = sum_i pw[co, i] * dw[2*(co//2)+i, j, t]
  - The instance norm is folded into weights + bias:
        lhsT[ci, co] = w_eff[co, j(ci), t] * scale[ci]
        bias'[co]    = bias[co] - sum_ci,t lhsT[ci, t, co] * mean[ci]
    where scale_c = 1/(sqrt(var_c)+eps); the padded border cells hold
    mean_c so that (border - mean)*scale = 0 matches the reference's
    zero-padded normalized input.
  - The grouped 3x3 conv runs on the TensorEngine as 9 shifted
    block-diagonal (2x2 blocks) 128x128 bf16 matmuls accumulated in PSUM,
    one pass per tap, channels on partitions (two halves of 128 channels).
  - Block-diag matrices: scatter the *unscaled* w_eff into a
    zero-initialized DRAM scratch (inline const) with strided DMAs (no
    stats dependency -> overlaps the x DMA-in), load dense [128,128]
    tiles back, then scale+cast per-partition (scale is indexed by ci =
    partition).  bias' comes from 9 accumulated N=1 matmuls of the scaled
    lhsT against mean[ci].
  - Per-half pipelining: half 0's conv overlaps half 1's input DMA.
"""

import sys

sys.path.insert(0, "/opt/trn_rl_repo")

from contextlib import ExitStack

import numpy as np
import ml_dtypes

from concourse import bacc, bass, mybir, tile
from concourse.bass_utils import run_bass_kernel_spmd

F32 = mybir.dt.float32
BF16 = mybir.dt.bfloat16
AX = mybir.AxisListType
OP = mybir.AluOpType
ACTF = mybir.ActivationFunctionType

C = 256          # channels (per sample)
H = W = 128      # spatial
P = 128          # partitions
HP = H + 2       # padded rows/cols (130)
NHF = 2          # channel halves
CHUNK_ROWS = 16  # rows per input DMA chunk
NCHUNK = H // CHUNK_ROWS          # 4 chunks per half
ROWS_PER_MM = 4                   # output rows per psum tile (4*128=512)
SB_TILES = 4                      # psum tiles per superblock
SB_ROWS = ROWS_PER_MM * SB_TILES  # 16 rows per superblock
NSB = H // SB_ROWS                # 8 superblocks per half
NPIX = H * W
EPS = 1e-7

_CACHED = {}


def build_nc():
    nc = bacc.Bacc(trn_type="TRN2")

    x_ext = nc.declare_dram_parameter("x", [C, H, W], F32, isOutput=False)
    dw_ext = nc.declare_dram_parameter("dw_kernels", [C, 2, 3, 3], F32, isOutput=False)
    pw_ext = nc.declare_dram_parameter("pw_kernels", [C, 2, 1, 1], F32, isOutput=False)
    b_ext = nc.declare_dram_parameter("biases", [C], F32, isOutput=False)
    out_ext = nc.declare_dram_parameter("out", [C, H, W], F32, isOutput=True)

    # zero-initialized DRAM scratch for the block-diag weight matrices;
    # runtime scatter only writes the (fixed) nonzero positions, so reuse
    # across executions is idempotent.  layout: [ci, hf, tap, co] f32
    # (ci-major so the load back to SBUF is one big descriptor per
    # partition instead of thousands of 512B ones)
    lhsT_dram = nc.inline_tensor(
        np.zeros((P, NHF, 9, P), dtype=ml_dtypes.bfloat16), name="lhsT_zero"
    )

    with tile.TileContext(nc) as tc, ExitStack() as ctx:
        const_pool = ctx.enter_context(tc.tile_pool(name="const", bufs=1))
        chunk_pool = ctx.enter_context(tc.tile_pool(name="chunk", bufs=8))
        sq_pool = ctx.enter_context(tc.tile_pool(name="sq", bufs=2))
        psum_pool = ctx.enter_context(tc.tile_pool(name="psum", bufs=8, space="PSUM"))
        stage_pool = ctx.enter_context(tc.tile_pool(name="stage", bufs=6))

        # ---------------- persistent tiles ----------------
        xnp = [
            const_pool.tile([P, HP, HP], BF16, name=f"xnp{hf}") for hf in range(NHF)
        ]
        sums = const_pool.tile([P, NHF, NCHUNK], F32, name="sums")
        sumsqs = const_pool.tile([P, NHF, NCHUNK], F32, name="sumsqs")

        mean_ch = const_pool.tile([P, NHF], F32, name="mean_ch")
        mean_bf = const_pool.tile([P, NHF], BF16, name="mean_bf")
        scale_ch = const_pool.tile([P, NHF], F32, name="scale_ch")
        bias_ch = const_pool.tile([P, NHF], F32, name="bias_ch")
        biasp_ch = const_pool.tile([P, NHF], F32, name="biasp_ch")
        st_a = const_pool.tile([P, NHF], F32, name="st_a")
        st_b = const_pool.tile([P, NHF], F32, name="st_b")

        # group-layout weights (partition = group)
        dwg = const_pool.tile([P, 2, 2, 9], F32, name="dwg")    # [g, i, j, t]
        pwg = const_pool.tile([P, 2, 2], F32, name="pwg")       # [g, o, i]
        weff = const_pool.tile([P, 2, 2, 9], F32, name="weff")  # [g, o, j, t]
        wpb = const_pool.tile([P, 2, 2, 9], BF16, name="wpb")   # weff bf16

        # dense block-diag weights: raw f32 (unscaled) and scaled bf16
        lhsT_raw = const_pool.tile([P, NHF, 9, P], BF16, name="lhsT_raw")
        lhsT_sb = const_pool.tile([P, NHF, 9, P], BF16, name="lhsT_sb")

        # dummy tiles to pre-warm the ScalarE LUT tables (Sqrt/Identity)
        # off the critical stats->scale chain (each lazy load is ~1.3us)
        zz = const_pool.tile([P, 1], F32, name="zz")
        zz2 = const_pool.tile([P, 1], F32, name="zz2")
        with tc.high_priority():
            nc.vector.memset(zz[:], 0.0)
            nc.scalar.sqrt(zz2[:], zz[:])
            nc.scalar.activation(
                out=zz2[:], in_=zz[:], func=ACTF.Identity, bias=zz[:], scale=0.0
            )

        # ------------- early DMAs (no stats dependency) -------------
        nc.sync.dma_start(
            out=dwg[:],
            in_=bass.AP(tensor=dw_ext, offset=0, ap=[[36, P], [18, 2], [9, 2], [1, 9]]),
        )
        nc.sync.dma_start(
            out=pwg[:],
            in_=bass.AP(tensor=pw_ext, offset=0, ap=[[4, P], [2, 2], [1, 2]]),
        )

        # ------------- w_eff (group layout) + scatter + load -------------
        # at high priority so the scatter->load chain completes early in
        # the x DMA-in window (the DVE stream would otherwise order these
        # after all the chunk conversions)
        with tc.high_priority():
            for o in range(2):
                nc.vector.tensor_scalar(
                    out=weff[:, o],
                    in0=dwg[:, 0],
                    scalar1=pwg[:, o, 0:1],
                    scalar2=None,
                    op0=OP.mult,
                )
                nc.vector.scalar_tensor_tensor(
                    out=weff[:, o],
                    in0=dwg[:, 1],
                    scalar=pwg[:, o, 1:2],
                    in1=weff[:, o],
                    op0=OP.mult,
                    op1=OP.add,
                )
            wpb_inst = nc.vector.tensor_copy(wpb[:], weff[:])
        # scatter: dst (ci=2a+j, hf, t, co=2a+o) <- weff[64*hf + a, o, j, t]
        # (DMA APs max out at 3 dims incl. the trailing unit -> one DMA
        #  per (hf, t, j) with dims (a, o))
        CI_STRIDE = NHF * 9 * P  # 2304

        def emit_scatter_load(hf, scatter_eng):
            # one DMA per (hf, t, j): dims (a, o) — 4-byte contiguous runs
            # (o-pairs), 2304 descriptors total, issued on a sequencer that
            # overlaps the x stream
            for t in range(9):
                for j in range(2):
                    scatter_eng.dma_start(
                        out=bass.AP(
                            tensor=lhsT_dram,
                            offset=j * CI_STRIDE + hf * 9 * P + t * P,
                            ap=[[2 * CI_STRIDE + 2, 64], [1, 2]],
                        ),
                        in_=wpb[64 * hf : 64 * (hf + 1), :, j, t],
                    )
            # load back densely: lhsT_raw[ci, hf, t, co] (contiguous
            # 2.3KB bf16 per partition)
            return nc.sync.dma_start(
                out=lhsT_raw[:, hf],
                in_=bass.AP(
                    tensor=lhsT_dram,
                    offset=hf * 9 * P,
                    ap=[[CI_STRIDE, P], [P, 9], [1, P]],
                ),
            )

        # x input chunks.  half 0 (latency-critical): split across both
        # HWDGE rings (SP + ACT) — ACT's stream is free pre-conv.  half 1:
        # SP ring only, so the ACT engine never blocks on DMA waits
        # mid-conv (that stalls epilogues -> PSUM banks -> TensorEngine).
        chunk_tiles = {0: [], 1: []}

        def emit_chunk(hf, ck):
            chv = chunk_pool.tile([P, CHUNK_ROWS, W], F32, name="chv")
            chunk_tiles[hf].append(chv)
            if hf == 0 and ck in (5, 7):
                dma_eng = nc.gpsimd
            elif hf == 0 and ck % 2 == 1:
                dma_eng = nc.scalar
            else:
                dma_eng = nc.sync
            return dma_eng.dma_start(
                out=chv[:],
                in_=x_ext[
                    hf * P : (hf + 1) * P,
                    ck * CHUNK_ROWS : (ck + 1) * CHUNK_ROWS,
                    :,
                ],
            )

        # ring/issue order: all h0 chunk issues first (so no chunk sits
        # behind the 18 scatter issues on the sync sequencer), then h0's
        # scatters+load.  h1's scatters+load run during conv h0.
        with tc.high_priority():
            for ck in range(NCHUNK):
                emit_chunk(0, ck)
            # h0 scatters issue from the ACT sequencer (free pre-conv);
            # their issue time overlaps the x stream, and the sync-ring
            # load waits on them without blocking x
            load0_inst = emit_scatter_load(0, nc.scalar)
            # bias [256] -> bias_ch[c, hf]
            nc.sync.dma_start(
                out=bias_ch[:],
                in_=bass.AP(tensor=b_ext, offset=0, ap=[[1, P], [P, NHF]]),
            )

        # ------------- per-half pipeline -------------
        for hf in range(NHF):
            if hf == 1:
                for ck in range(NCHUNK):
                    inst = emit_chunk(1, ck)
                    if ck == 0:
                        # keep h1's 8 MiB off the DMA slots until h0's
                        # latency-critical weight load has completed
                        bass._add_dep_helper(
                            inst.ins,
                            load0_inst.ins,
                            sync=True,
                            reason="h1 x stream waits for h0 lhsT load",
                        )
                emit_scatter_load(1, nc.sync)
            for ck in range(NCHUNK):
                chv = chunk_tiles[hf][ck]
                # convert f32 -> bf16 into padded interior; accumulate sum
                conv_inst = nc.vector.tensor_scalar(
                    out=xnp[hf][
                        :, 1 + ck * CHUNK_ROWS : 1 + (ck + 1) * CHUNK_ROWS, 1 : 1 + W
                    ],
                    in0=chv[:],
                    scalar1=1.0,
                    scalar2=None,
                    op0=OP.mult,
                    op1=OP.add,
                    accum_out=sums[:, hf, ck : ck + 1],
                )
                if hf == 0 and ck == 0:
                    # pin the DVE stream order: weff/wpb (deps land ~5us)
                    # must precede the conversions, else the scheduler may
                    # slot them mid-stream and stall the weight chain
                    bass._add_dep_helper(
                        conv_inst.ins,
                        wpb_inst.ins,
                        sync=True,
                        reason="weff/wpb before chunk conversions on DVE",
                    )
                # sum of squares via ScalarE
                sq = sq_pool.tile([P, CHUNK_ROWS, W], F32, name="sq")
                nc.scalar.activation(
                    out=sq[:],
                    in_=chv[:],
                    func=ACTF.Square,
                    accum_out=sumsqs[:, hf, ck : ck + 1],
                )

            # --- stats finalize (channel layout) ---
            nc.vector.tensor_reduce(
                out=st_a[:, hf : hf + 1], in_=sums[:, hf, :], axis=AX.X, op=OP.add
            )
            nc.vector.tensor_scalar(
                out=mean_ch[:, hf : hf + 1],
                in0=st_a[:, hf : hf + 1],
                scalar1=1.0 / NPIX,
                scalar2=None,
                op0=OP.mult,
            )
            nc.vector.tensor_reduce(
                out=st_a[:, hf : hf + 1], in_=sumsqs[:, hf, :], axis=AX.X, op=OP.add
            )
            nc.vector.tensor_tensor(
                out=st_b[:, hf : hf + 1],
                in0=mean_ch[:, hf : hf + 1],
                in1=mean_ch[:, hf : hf + 1],
                op=OP.mult,
            )
            nc.vector.scalar_tensor_tensor(
                out=st_b[:, hf : hf + 1],
                in0=st_b[:, hf : hf + 1],
                scalar=float(-NPIX),
                in1=st_a[:, hf : hf + 1],
                op0=OP.mult,
                op1=OP.add,
            )
            nc.vector.tensor_scalar(
                out=st_b[:, hf : hf + 1],
                in0=st_b[:, hf : hf + 1],
                scalar1=1.0 / (NPIX - 1),
                scalar2=None,
                op0=OP.mult,
            )
            nc.scalar.sqrt(st_b[:, hf : hf + 1], st_b[:, hf : hf + 1])
            nc.vector.tensor_scalar(
                out=st_b[:, hf : hf + 1],
                in0=st_b[:, hf : hf + 1],
                scalar1=EPS,
                scalar2=None,
                op0=OP.add,
            )
            nc.vector.reciprocal(scale_ch[:, hf : hf + 1], st_b[:, hf : hf + 1])
            nc.vector.tensor_copy(mean_bf[:, hf : hf + 1], mean_ch[:, hf : hf + 1])

            # --- scale + cast the block-diag weights (per-partition ci) ---
            nc.vector.tensor_scalar(
                out=lhsT_sb[:, hf],
                in0=lhsT_raw[:, hf],
                scalar1=scale_ch[:, hf : hf + 1],
                scalar2=None,
                op0=OP.mult,
            )

            # --- bias' = bias - lhsT^T @ mean  (9 accumulated N=1 matmuls) ---
            bps = psum_pool.tile([P, 1], F32, name="bps", tag="ps", bufs=8)
            for t in range(9):
                nc.tensor.matmul(
                    bps[:],
                    lhsT=lhsT_sb[:, hf, t, :],
                    rhs=mean_bf[:, hf : hf + 1],
                    start=(t == 0),
                    stop=(t == 8),
                )
            nc.vector.tensor_tensor(
                out=biasp_ch[:, hf : hf + 1],
                in0=bias_ch[:, hf : hf + 1],
                in1=bps[:],
                op=OP.subtract,
            )

            # --- border fill with mean (bf16) ---
            bias_ap = mean_ch[:, hf : hf + 1]
            nc.scalar.activation(
                out=xnp[hf][:, 1 : 1 + H, 0],
                in_=xnp[hf][:, 1 : 1 + H, 1],
                func=ACTF.Identity,
                bias=bias_ap,
                scale=0.0,
            )
            nc.scalar.activation(
                out=xnp[hf][:, 1 : 1 + H, HP - 1],
                in_=xnp[hf][:, 1 : 1 + H, 1],
                func=ACTF.Identity,
                bias=bias_ap,
                scale=0.0,
            )
            nc.scalar.activation(
                out=xnp[hf][:, 0, :],
                in_=xnp[hf][:, 1, :],
                func=ACTF.Identity,
                bias=bias_ap,
                scale=0.0,
            )
            nc.scalar.activation(
                out=xnp[hf][:, HP - 1, :],
                in_=xnp[hf][:, 1, :],
                func=ACTF.Identity,
                bias=bias_ap,
                scale=0.0,
            )

            # --- conv: 9 shifted block-diag matmuls per psum tile ---
            for sb in range(NSB):
                ps = [
                    psum_pool.tile([P, ROWS_PER_MM, W], F32, name="ps", tag="ps", bufs=8)
                    for _ in range(SB_TILES)
                ]
                for t in range(9):
                    dy, dx = t // 3, t % 3
                    for k in range(SB_TILES):
                        h0 = sb * SB_ROWS + k * ROWS_PER_MM
                        nc.tensor.matmul(
                            ps[k][:],
                            lhsT=lhsT_sb[:, hf, t, :],
                            rhs=xnp[hf][
                                :, h0 + dy : h0 + dy + ROWS_PER_MM, dx : dx + W
                            ],
                            start=(t == 0),
                            stop=(t == 8),
                        )
                # epilogue + store in 8-row blocks (2 psum tiles each) to
                # keep the kernel tail short
                for half_blk in range(2):
                    stg = stage_pool.tile([P, SB_ROWS // 2, W], F32, name="stg")
                    for kk in range(2):
                        k = half_blk * 2 + kk
                        nc.scalar.activation(
                            out=stg[:, kk * ROWS_PER_MM : (kk + 1) * ROWS_PER_MM, :],
                            in_=ps[k][:],
                            func=ACTF.Identity,
                            bias=biasp_ch[:, hf : hf + 1],
                            scale=1.0,
                        )
                    nc.sync.dma_start(
                        out=out_ext[
                            hf * P : (hf + 1) * P,
                            sb * SB_ROWS
                            + half_blk * (SB_ROWS // 2) : sb * SB_ROWS
                            + (half_blk + 1) * (SB_ROWS // 2),
                            :,
                        ],
                        in_=stg[:],
                    )

    nc.compile()
    return nc


def get_nc():
    if "nc" not in _CACHED:
        _CACHED["nc"] = build_nc()
    return _CACHED["nc"]


def kernel(x, dw_kernels, pw_kernels, biases):
    x = np.asarray(x, dtype=np.float32)
    dw_kernels = np.asarray(dw_kernels, dtype=np.float32)
    pw_kernels = np.asarray(pw_kernels, dtype=np.float32)
    biases = np.asarray(biases, dtype=np.float32)
    B = x.shape[0]
    assert B == 8

    nc = get_nc()
    in_maps = [
        {
            "x": np.ascontiguousarray(x[i]),
            "dw_kernels": np.ascontiguousarray(dw_kernels[i]),
            "pw_kernels": np.ascontiguousarray(pw_kernels[i]),
            "biases": np.ascontiguousarray(biases[i]),
        }
        for i in range(B)
    ]
    res = run_bass_kernel_spmd(nc, in_maps, core_ids=list(range(B)))
    return np.stack([res.results[i]["out"] for i in range(B)], axis=0)



# revision 7
# speedup vs baseline: 1.1125x; 1.1125x over previous
"""AdaConv2D Trainium2 Bass kernel (fp8-DoubleRow + bf16 hybrid conv).

Problem (per sample): instance-norm(x) -> grouped 3x3 conv (128 groups,
2ch/group, per-sample weights) -> grouped 1x1 conv -> +bias.
B=8, Cin=Cout=256, H=W=128.  Pure data-parallel: 1 sample per NeuronCore.

Math: the 1x1 conv folds into the 3x3 taps (w_eff), the instance norm
folds into the weights (scale per in-channel ci) and bias:
    out = W_s @ x_pad + bias',   W_s[ci,t,co] = w_eff * S/(std_ci+eps)
    bias'[co] = bias[co] - (sum_{ci,t} W_s * mean_ci)/S
with x_pad borders held at mean_ci so border windows cancel, and a
global S=128 pre-scale so fp8-quantized weights stay in e4m3's normal
range (the epilogue multiplies by 1/S).

Precision/speed plan (validated vs f64 reference, ~1.5% L2 global):
  - Host sends x as an fp8 e4m3 hi/lo pair (xh=fp8(x), xl=fp8(x-xh)),
    pre-padded to 130x130 -> 8.5 MiB/core input instead of 16.
  - taps 0..5 run as 3 fp8 DoubleRow matmuls (2 taps per instruction,
    both k-tiles read the hi plane) -- DR costs the same per instruction
    as one bf16 matmul but does 2 taps.
  - taps 6..8 run as bf16 matmuls against xb = bf16(xh+xl) (full
    precision), so the quantization error stays ~1.5x under the 2e-2
    correctness gate.
  -> 6 PE instructions per psum tile instead of 9 (bf16-only).
  - Output is written bf16 (8 MiB) and upcast to f32 on the host.

Per-core dataflow:
  - x streams in 10 chunks/half; per chunk DVE adds hi+lo -> xb (bf16),
    GPSIMD reduces xb for sums, ACT squares the hi plane for sumsq.
  - w_eff scatters (via a zero DRAM scratch) into dense block-diag
    [ci, tap, co] layout, loaded back before stats land; after stats a
    DVE pass scales by S/std and quantizes to the fp8/bf16 lhsT tiles.
  - bias' comes from 6 accumulated N=1 matmuls against the fp8/bf16
    mean, mirroring the conv arithmetic exactly (border cancellation).
  - conv: per 16-row superblock, 4 psum tiles x 6 slot-instructions;
    ACT epilogue applies 1/S + bias' and emits bf16; DMA out.
"""

import sys

sys.path.insert(0, "/opt/trn_rl_repo")

from contextlib import ExitStack

import numpy as np
import ml_dtypes

from concourse import bacc, bass, mybir, tile
from concourse.bass_utils import run_bass_kernel_spmd

F32 = mybir.dt.float32
BF16 = mybir.dt.bfloat16
FP8 = mybir.dt.float8e4
AX = mybir.AxisListType
OP = mybir.AluOpType
ACTF = mybir.ActivationFunctionType
DR = mybir.MatmulPerfMode.DoubleRow

C = 256          # channels (per sample)
H = W = 128      # spatial
P = 128          # partitions
HP = H + 2       # padded rows/cols (130)
NHF = 2          # channel halves
NCHUNK = 10      # input DMA chunks per half (13 padded rows each)
CHUNK_TR = HP // NCHUNK           # 13 tile rows per chunk
ROWS_PER_MM = 4                   # output rows per psum tile (4*128=512)
SB_TILES = 4                      # psum tiles per superblock
SB_ROWS = ROWS_PER_MM * SB_TILES  # 16 rows per superblock
NSB = H // SB_ROWS                # 8 superblocks per half
NPIX = H * W
EPS = 1e-7
S = 128.0        # weight pre-scale (fp8 range), undone in the epilogue

TAPS = [(t // 3, t % 3) for t in range(9)]
FP8_PAIRS = [(0, 1), (2, 3), (4, 5)]  # DoubleRow tap pairs (hi plane)
BF16_TAPS = [6, 7, 8]                 # bf16 taps (xb plane)
NPAIR = len(FP8_PAIRS)
NB16 = len(BF16_TAPS)
NSLOT = NPAIR + NB16

_CACHED = {}


def build_nc():
    nc = bacc.Bacc(trn_type="TRN2")

    xpad_ext = nc.declare_dram_parameter("xpad", [C, 2, HP, HP], FP8, isOutput=False)
    dw_ext = nc.declare_dram_parameter("dw_kernels", [C, 2, 3, 3], F32, isOutput=False)
    pw_ext = nc.declare_dram_parameter("pw_kernels", [C, 2, 1, 1], F32, isOutput=False)
    b_ext = nc.declare_dram_parameter("biases", [C], F32, isOutput=False)
    out_ext = nc.declare_dram_parameter("out", [C, H, W], BF16, isOutput=True)

    # zero-initialized DRAM scratch for the dense block-diag w_eff
    # (runtime scatter only writes the fixed nonzero slots -> idempotent).
    # layout [ci, hf, tap, co] f32
    weff_dram = nc.inline_tensor(
        np.zeros((P, NHF, 9, P), dtype=np.float32), name="weff_zero"
    )
    CI_STRIDE = NHF * 9 * P  # 2304 elements per ci row

    with tile.TileContext(nc) as tc, ExitStack() as ctx:
        const_pool = ctx.enter_context(tc.tile_pool(name="const", bufs=1))
        sq_pool = ctx.enter_context(tc.tile_pool(name="sq", bufs=4))
        psum_pool = ctx.enter_context(tc.tile_pool(name="psum", bufs=8, space="PSUM"))
        stage_pool = ctx.enter_context(tc.tile_pool(name="stage", bufs=6))

        # ---------------- persistent tiles ----------------
        xpt = [const_pool.tile([P, 2, HP, HP], FP8, name=f"xpt{hf}") for hf in range(NHF)]
        xbt = [const_pool.tile([P, HP, HP], BF16, name=f"xbt{hf}") for hf in range(NHF)]

        sums = const_pool.tile([P, NHF, NCHUNK], F32, name="sums")
        sumsqs = const_pool.tile([P, NHF, NCHUNK], F32, name="sumsqs")
        st_a = const_pool.tile([P, NHF], F32, name="st_a")
        st_b = const_pool.tile([P, NHF], F32, name="st_b")
        st_c = const_pool.tile([P, NHF], F32, name="st_c")
        mean_ch = const_pool.tile([P, NHF], F32, name="mean_ch")
        mean_bf = const_pool.tile([P, NHF], BF16, name="mean_bf")
        mqt = const_pool.tile([P, NHF, 2, 1], FP8, name="mqt")
        scS = const_pool.tile([P, NHF], F32, name="scS")
        bias_ch = const_pool.tile([P, NHF], F32, name="bias_ch")
        biasp_ch = const_pool.tile([P, NHF], F32, name="biasp_ch")

        # group-layout weights (partition = group)
        dwg = const_pool.tile([P, 2, 2, 9], F32, name="dwg")    # [g, i, j, t]
        pwg = const_pool.tile([P, 2, 2], F32, name="pwg")       # [g, o, i]
        weffg = const_pool.tile([P, 2, 2, 9], F32, name="weffg")  # [g, o, j, t]

        # dense block-diag weights
        weffd = const_pool.tile([P, NHF, 9, P], F32, name="weffd")   # unscaled
        wtmp = const_pool.tile([P, NHF, 9, P], F32, name="wtmp")     # S*scaled
        wf8 = const_pool.tile([P, NHF, NPAIR, 2, P], FP8, name="wf8")
        wb16 = const_pool.tile([P, NHF, NB16, P], BF16, name="wb16")

        # ACT LUT warm (sqrt/square/identity) off the critical chains
        zz = const_pool.tile([P, 1], F32, name="zz")
        zz2 = const_pool.tile([P, 1], F32, name="zz2")
        with tc.high_priority():
            nc.vector.memset(zz[:], 0.0)
            nc.scalar.activation(out=zz2[:], in_=zz[:], func=ACTF.Square)
            nc.scalar.sqrt(zz2[:], zz[:])
            nc.scalar.activation(
                out=zz2[:], in_=zz[:], func=ACTF.Identity, bias=zz[:], scale=0.0
            )

        # ------------- early DMAs (no stats dependency) -------------
        nc.sync.dma_start(
            out=dwg[:],
            in_=bass.AP(tensor=dw_ext, offset=0, ap=[[36, P], [18, 2], [9, 2], [1, 9]]),
        )
        nc.sync.dma_start(
            out=pwg[:],
            in_=bass.AP(tensor=pw_ext, offset=0, ap=[[4, P], [2, 2], [1, 2]]),
        )

        # ------------- w_eff (group layout) + scatter + load -------------
        with tc.high_priority():
            for o in range(2):
                nc.vector.tensor_scalar(
                    out=weffg[:, o],
                    in0=dwg[:, 0],
                    scalar1=pwg[:, o, 0:1],
                    scalar2=None,
                    op0=OP.mult,
                )
                nc.vector.scalar_tensor_tensor(
                    out=weffg[:, o],
                    in0=dwg[:, 1],
                    scalar=pwg[:, o, 1:2],
                    in1=weffg[:, o],
                    op0=OP.mult,
                    op1=OP.add,
                )

        def emit_scatter_load(hf, scatter_eng):
            # scatter: dst (ci=2a+j, hf, t, co=2a+o) <- weffg[64*hf + a, o, j, t]
            for t in range(9):
                for j in range(2):
                    scatter_eng.dma_start(
                        out=bass.AP(
                            tensor=weff_dram,
                            offset=j * CI_STRIDE + hf * 9 * P + t * P,
                            ap=[[2 * CI_STRIDE + 2, 64], [1, 2]],
                        ),
                        in_=weffg[64 * hf : 64 * (hf + 1), :, j, t],
                    )
            # dense load back: weffd[ci, hf, t, co]
            return nc.sync.dma_start(
                out=weffd[:, hf],
                in_=bass.AP(
                    tensor=weff_dram,
                    offset=hf * 9 * P,
                    ap=[[CI_STRIDE, P], [P, 9], [1, P]],
                ),
            )

        # ------------- x input chunks -------------
        def emit_chunk(hf, ck):
            r0 = ck * CHUNK_TR
            r1 = r0 + CHUNK_TR
            if hf == 0 and ck % 3 == 1:
                dma_eng = nc.scalar
            elif hf == 0 and ck % 3 == 2:
                dma_eng = nc.gpsimd
            else:
                dma_eng = nc.sync
            return dma_eng.dma_start(
                out=xpt[hf][:, :, r0:r1, :],
                in_=bass.AP(
                    tensor=xpad_ext,
                    offset=hf * P * 2 * HP * HP + r0 * HP,
                    ap=[[2 * HP * HP, P], [HP * HP, 2], [1, CHUNK_TR * HP]],
                ),
            )

        with tc.high_priority():
            for ck in range(NCHUNK):
                emit_chunk(0, ck)
            load0_inst = emit_scatter_load(0, nc.scalar)
            nc.sync.dma_start(
                out=bias_ch[:],
                in_=bass.AP(tensor=b_ext, offset=0, ap=[[1, P], [P, NHF]]),
            )

        # ------------- per-half pipeline -------------
        for hf in range(NHF):
            if hf == 1:
                for ck in range(NCHUNK):
                    inst = emit_chunk(1, ck)
                    if ck == 0:
                        bass._add_dep_helper(
                            inst.ins,
                            load0_inst.ins,
                            sync=True,
                            reason="h1 x stream waits for h0 weight load",
                        )
                emit_scatter_load(1, nc.sync)

            # --- per-chunk: xb = hi+lo (DVE), sums (GPSIMD), sumsq (ACT) ---
            for ck in range(NCHUNK):
                r0 = max(1, ck * CHUNK_TR)
                r1 = min(1 + H, (ck + 1) * CHUNK_TR)
                nc.vector.tensor_tensor(
                    out=xbt[hf][:, r0:r1, 1 : 1 + W],
                    in0=xpt[hf][:, 0, r0:r1, 1 : 1 + W],
                    in1=xpt[hf][:, 1, r0:r1, 1 : 1 + W],
                    op=OP.add,
                )
                gtr = sq_pool.tile([P, CHUNK_TR, W], BF16, name="gtr")
                nc.vector.tensor_scalar(
                    out=gtr[:, 0 : r1 - r0, :],
                    in0=xbt[hf][:, r0:r1, 1 : 1 + W],
                    scalar1=1.0,
                    scalar2=None,
                    op0=OP.mult,
                    op1=OP.add,
                    accum_out=sums[:, hf, ck : ck + 1],
                )
                sq = sq_pool.tile([P, CHUNK_TR, W], BF16, name="sq")
                nc.scalar.activation(
                    out=sq[:, 0 : r1 - r0, :],
                    in_=xpt[hf][:, 0, r0:r1, 1 : 1 + W],
                    func=ACTF.Square,
                    accum_out=sumsqs[:, hf, ck : ck + 1],
                )

            # --- stats finalize ---
            with tc.high_priority():
                nc.vector.tensor_reduce(
                    out=st_a[:, hf : hf + 1], in_=sums[:, hf, :], axis=AX.X, op=OP.add
                )
                nc.vector.tensor_scalar(
                    out=mean_ch[:, hf : hf + 1],
                    in0=st_a[:, hf : hf + 1],
                    scalar1=1.0 / NPIX,
                    scalar2=None,
                    op0=OP.mult,
                )
                nc.vector.tensor_reduce(
                    out=st_b[:, hf : hf + 1], in_=sumsqs[:, hf, :], axis=AX.X, op=OP.add
                )
                nc.vector.tensor_tensor(
                    out=st_c[:, hf : hf + 1],
                    in0=mean_ch[:, hf : hf + 1],
                    in1=mean_ch[:, hf : hf + 1],
                    op=OP.mult,
                )
                nc.vector.scalar_tensor_tensor(
                    out=st_b[:, hf : hf + 1],
                    in0=st_c[:, hf : hf + 1],
                    scalar=float(-NPIX),
                    in1=st_b[:, hf : hf + 1],
                    op0=OP.mult,
                    op1=OP.add,
                )
                nc.vector.tensor_scalar(
                    out=st_b[:, hf : hf + 1],
                    in0=st_b[:, hf : hf + 1],
                    scalar1=1.0 / (NPIX - 1),
                    scalar2=None,
                    op0=OP.mult,
                )
                nc.scalar.sqrt(st_b[:, hf : hf + 1], st_b[:, hf : hf + 1])
                # (std + EPS) / S, then reciprocal -> S/(std+EPS)
                nc.vector.tensor_scalar(
                    out=st_b[:, hf : hf + 1],
                    in0=st_b[:, hf : hf + 1],
                    scalar1=EPS,
                    scalar2=1.0 / S,
                    op0=OP.add,
                    op1=OP.mult,
                )
                nc.vector.reciprocal(scS[:, hf : hf + 1], st_b[:, hf : hf + 1])
                nc.vector.tensor_copy(mean_bf[:, hf : hf + 1], mean_ch[:, hf : hf + 1])
                nc.vector.tensor_copy(mqt[:, hf, 0], mean_ch[:, hf : hf + 1])
                nc.vector.tensor_copy(mqt[:, hf, 1], mean_ch[:, hf : hf + 1])

                # --- scale + quantize the dense weights ---
                nc.vector.tensor_scalar(
                    out=wtmp[:, hf],
                    in0=weffd[:, hf],
                    scalar1=scS[:, hf : hf + 1],
                    scalar2=None,
                    op0=OP.mult,
                )
                # fp8 taps 0..5 -> wf8[hf] ([P, NPAIR*2, P] contiguous)
                nc.vector.tensor_copy(
                    bass.AP(
                        tensor=wf8[:].tensor,
                        offset=hf * NPAIR * 2 * P,
                        ap=[[NHF * NPAIR * 2 * P, P], [P, NPAIR * 2], [1, P]],
                    ),
                    wtmp[:, hf, 0 : 2 * NPAIR, :],
                )
                # bf16 taps 6..8
                nc.vector.tensor_copy(wb16[:, hf], wtmp[:, hf, 2 * NPAIR : 9, :])

            # --- border fills (bias = mean, scale = 0) ---
            bias_ap = mean_ch[:, hf : hf + 1]
            with tc.high_priority():
                for tgt, dt_ in ((xbt[hf], BF16), (None, FP8)):
                    if tgt is None:
                        # hi plane of xpt: only the fp8 taps read it
                        def edge(sl):
                            return xpt[hf][(slice(None), 0) + sl]
                    else:
                        def edge(sl):
                            return tgt[(slice(None),) + sl]
                    nc.scalar.activation(
                        out=edge((slice(1, 1 + H), 0)),
                        in_=edge((slice(1, 1 + H), 1)),
                        func=ACTF.Identity, bias=bias_ap, scale=0.0,
                    )
                    nc.scalar.activation(
                        out=edge((slice(1, 1 + H), HP - 1)),
                        in_=edge((slice(1, 1 + H), 1)),
                        func=ACTF.Identity, bias=bias_ap, scale=0.0,
                    )
                    nc.scalar.activation(
                        out=edge((0, slice(None))),
                        in_=edge((1, slice(None))),
                        func=ACTF.Identity, bias=bias_ap, scale=0.0,
                    )
                    nc.scalar.activation(
                        out=edge((HP - 1, slice(None))),
                        in_=edge((1, slice(None))),
                        func=ACTF.Identity, bias=bias_ap, scale=0.0,
                    )

            # --- bias' = bias - (W_s @ mean)/S  (6 accumulated N=1 matmuls) ---
            bps = psum_pool.tile([P, 1], F32, name="bps", tag="ps", bufs=8)
            si = 0
            for p in range(NPAIR):
                nc.tensor.matmul(
                    bps[:],
                    lhsT=wf8[:, hf, p],
                    rhs=mqt[:, hf],
                    start=(si == 0),
                    stop=(si == NSLOT - 1),
                    perf_mode=DR,
                )
                si += 1
            for i in range(NB16):
                nc.tensor.matmul(
                    bps[:],
                    lhsT=wb16[:, hf, i],
                    rhs=mean_bf[:, hf : hf + 1],
                    start=(si == 0),
                    stop=(si == NSLOT - 1),
                )
                si += 1
            nc.vector.scalar_tensor_tensor(
                out=biasp_ch[:, hf : hf + 1],
                in0=bps[:],
                scalar=-1.0 / S,
                in1=bias_ch[:, hf : hf + 1],
                op0=OP.mult,
                op1=OP.add,
            )

            # --- conv: per superblock, 6 slot-instructions x 4 psum tiles ---
            XPITCH = 2 * HP * HP  # xpt partition pitch (elements)
            for sb in range(NSB):
                ps = [
                    psum_pool.tile([P, ROWS_PER_MM, W], F32, name="ps", tag="ps", bufs=8)
                    for _ in range(SB_TILES)
                ]
                si = 0
                for p, (t0, t1) in enumerate(FP8_PAIRS):
                    dy0, dx0 = TAPS[t0]
                    dy1, dx1 = TAPS[t1]
                    delta = (dy1 - dy0) * HP + (dx1 - dx0)
                    for k in range(SB_TILES):
                        h0 = sb * SB_ROWS + k * ROWS_PER_MM
                        rhs = bass.AP(
                            tensor=xpt[hf][:].tensor,
                            offset=(h0 + dy0) * HP + dx0,
                            ap=[[XPITCH, P], [delta, 2], [HP, ROWS_PER_MM], [1, W]],
                        )
                        nc.tensor.matmul(
                            ps[k][:],
                            lhsT=wf8[:, hf, p],
                            rhs=rhs,
                            start=(si == 0),
                            stop=(si == NSLOT - 1),
                            perf_mode=DR,
                        )
                    si += 1
                for i, t in enumerate(BF16_TAPS):
                    dy, dx = TAPS[t]
                    for k in range(SB_TILES):
                        h0 = sb * SB_ROWS + k * ROWS_PER_MM
                        nc.tensor.matmul(
                            ps[k][:],
                            lhsT=wb16[:, hf, i],
                            rhs=xbt[hf][:, h0 + dy : h0 + dy + ROWS_PER_MM, dx : dx + W],
                            start=(si == 0),
                            stop=(si == NSLOT - 1),
                        )
                    si += 1
                # epilogue + store in 8-row blocks (2 psum tiles each)
                for half_blk in range(2):
                    stg = stage_pool.tile([P, SB_ROWS // 2, W], BF16, name="stg")
                    for kk in range(2):
                        k = half_blk * 2 + kk
                        nc.scalar.activation(
                            out=stg[:, kk * ROWS_PER_MM : (kk + 1) * ROWS_PER_MM, :],
                            in_=ps[k][:],
                            func=ACTF.Identity,
                            bias=biasp_ch[:, hf : hf + 1],
                            scale=1.0 / S,
                        )
                    nc.sync.dma_start(
                        out=out_ext[
                            hf * P : (hf + 1) * P,
                            sb * SB_ROWS
                            + half_blk * (SB_ROWS // 2) : sb * SB_ROWS
                            + (half_blk + 1) * (SB_ROWS // 2),
                            :,
                        ],
                        in_=stg[:],
                    )

    nc.compile()
    return nc


def get_nc():
    if "nc" not in _CACHED:
        _CACHED["nc"] = build_nc()
    return _CACHED["nc"]


def make_in_maps(x, dw_kernels, pw_kernels, biases):
    x = np.asarray(x, dtype=np.float32)
    dw_kernels = np.asarray(dw_kernels, dtype=np.float32)
    pw_kernels = np.asarray(pw_kernels, dtype=np.float32)
    biases = np.asarray(biases, dtype=np.float32)
    B = x.shape[0]
    in_maps = []
    for i in range(B):
        xh = x[i].astype(ml_dtypes.float8_e4m3)
        xl = (x[i] - xh.astype(np.float32)).astype(ml_dtypes.float8_e4m3)
        xpad = np.zeros((C, 2, HP, HP), dtype=ml_dtypes.float8_e4m3)
        xpad[:, 0, 1 : 1 + H, 1 : 1 + W] = xh
        xpad[:, 1, 1 : 1 + H, 1 : 1 + W] = xl
        in_maps.append(
            {
                "xpad": xpad,
                "dw_kernels": np.ascontiguousarray(dw_kernels[i]),
                "pw_kernels": np.ascontiguousarray(pw_kernels[i]),
                "biases": np.ascontiguousarray(biases[i]),
            }
        )
    return in_maps


def postprocess(res, B):
    return np.stack(
        [np.asarray(res.results[i]["out"]).astype(np.float32) for i in range(B)], axis=0
    )


def kernel(x, dw_kernels, pw_kernels, biases):
    B = np.asarray(x).shape[0]
    assert B == 8
    nc = get_nc()
    in_maps = make_in_maps(x, dw_kernels, pw_kernels, biases)
    res = run_bass_kernel_spmd(nc, in_maps, core_ids=list(range(B)))
    return postprocess(res, B)


# revision 8
# speedup vs baseline: 1.1452x; 1.0293x over previous
"""AdaConv2D Trainium2 Bass kernel (fp8-DoubleRow + bf16 hybrid conv).

Problem (per sample): instance-norm(x) -> grouped 3x3 conv (128 groups,
2ch/group, per-sample weights) -> grouped 1x1 conv -> +bias.
B=8, Cin=Cout=256, H=W=128.  Pure data-parallel: 1 sample per NeuronCore.

Math: the 1x1 conv folds into the 3x3 taps (w_eff), the instance norm
folds into the weights (scale per in-channel ci) and bias:
    out = W_s @ x_pad + bias',   W_s[ci,t,co] = w_eff * S/(std_ci+eps)
    bias'[co] = bias[co] - (sum_{ci,t} W_s * mean_ci)/S
with x_pad borders held at mean_ci so border windows cancel, and a
global S=128 pre-scale so fp8-quantized weights stay in e4m3's normal
range (the epilogue multiplies by 1/S).

Precision/speed plan (validated vs f64 reference, ~1.5% L2 global,
gate is 2e-2):
  - Host sends x twice, pre-padded to 130x130: xq = fp8 e4m3 (4.3 MiB)
    and xb = bf16 (8.7 MiB).
  - taps 0..5 run as 3 fp8 DoubleRow matmuls on xq (2 taps per
    instruction; DR costs the same per instruction as one bf16 matmul
    but does 2 taps).
  - taps 6..8 run as bf16 matmuls on xb (near-full precision).
  -> 6 PE instructions per psum tile instead of 9 (bf16-only).
  - Output is written bf16 (8 MiB) and upcast to f32 on the host.

Per-core dataflow:
  - xb streams first (10 chunks/half): DVE accumulates sums, ACT
    accumulates sum-of-squares; the xq stream rides the DMA tail.
  - w_eff scatters (via a zero DRAM scratch) into dense block-diag
    [ci, tap, co] layout, loaded back before stats land; after stats a
    DVE pass scales by S/std and quantizes to the fp8/bf16 lhsT tiles.
  - bias' comes from 6 accumulated N=1 matmuls against the fp8/bf16
    mean, mirroring the conv arithmetic exactly (border cancellation).
  - conv: per 16-row superblock, 4 psum tiles x 6 slot-instructions;
    epilogues alternate ACT/DVE (1/S scale + bias'), emit bf16, DMA out.
"""

import sys

sys.path.insert(0, "/opt/trn_rl_repo")

from contextlib import ExitStack

import numpy as np
import ml_dtypes

from concourse import bacc, bass, mybir, tile
from concourse.bass_utils import run_bass_kernel_spmd

F32 = mybir.dt.float32
BF16 = mybir.dt.bfloat16
FP8 = mybir.dt.float8e4
AX = mybir.AxisListType
OP = mybir.AluOpType
ACTF = mybir.ActivationFunctionType
DR = mybir.MatmulPerfMode.DoubleRow

C = 256          # channels (per sample)
H = W = 128      # spatial
P = 128          # partitions
HP = H + 2       # padded rows/cols (130)
NHF = 2          # channel halves
NCHUNK = 10      # input DMA chunks per half (13 padded rows each)
CHUNK_TR = HP // NCHUNK           # 13 tile rows per chunk
ROWS_PER_MM = 4                   # output rows per psum tile (4*128=512)
SB_TILES = 4                      # psum tiles per superblock
SB_ROWS = ROWS_PER_MM * SB_TILES  # 16 rows per superblock
NSB = H // SB_ROWS                # 8 superblocks per half
NPIX = H * W
EPS = 1e-7
S = 128.0        # weight pre-scale (fp8 range), undone in the epilogue

TAPS = [(t // 3, t % 3) for t in range(9)]
FP8_PAIRS = [(0, 1), (2, 3), (4, 5)]  # DoubleRow tap pairs (xq)
BF16_TAPS = [6, 7, 8]                 # bf16 taps (xb)
NPAIR = len(FP8_PAIRS)
NB16 = len(BF16_TAPS)
NSLOT = NPAIR + NB16

_CACHED = {}


def build_nc():
    nc = bacc.Bacc(trn_type="TRN2")

    xq_ext = nc.declare_dram_parameter("xq", [C, HP, HP], FP8, isOutput=False)
    xb_ext = nc.declare_dram_parameter("xb", [C, HP, HP], BF16, isOutput=False)
    dw_ext = nc.declare_dram_parameter("dw_kernels", [C, 2, 3, 3], F32, isOutput=False)
    pw_ext = nc.declare_dram_parameter("pw_kernels", [C, 2, 1, 1], F32, isOutput=False)
    b_ext = nc.declare_dram_parameter("biases", [C], F32, isOutput=False)
    out_ext = nc.declare_dram_parameter("out", [C, H, W], BF16, isOutput=True)

    # zero-initialized DRAM scratch for the dense block-diag w_eff
    # (runtime scatter only writes the fixed nonzero slots -> idempotent).
    # layout [ci, hf, tap, co] f32
    weff_dram = nc.inline_tensor(
        np.zeros((P, NHF, 9, P), dtype=np.float32), name="weff_zero"
    )
    CI_STRIDE = NHF * 9 * P  # 2304 elements per ci row

    with tile.TileContext(nc) as tc, ExitStack() as ctx:
        const_pool = ctx.enter_context(tc.tile_pool(name="const", bufs=1))
        sq_pool = ctx.enter_context(tc.tile_pool(name="sq", bufs=4))
        psum_pool = ctx.enter_context(tc.tile_pool(name="psum", bufs=8, space="PSUM"))
        stage_pool = ctx.enter_context(tc.tile_pool(name="stage", bufs=6))

        # ---------------- persistent tiles ----------------
        xqt = [const_pool.tile([P, HP, HP], FP8, name=f"xqt{hf}") for hf in range(NHF)]
        xbt = [const_pool.tile([P, HP, HP], BF16, name=f"xbt{hf}") for hf in range(NHF)]

        sums = const_pool.tile([P, NHF, NCHUNK], F32, name="sums")
        sumsqs = const_pool.tile([P, NHF, NCHUNK], F32, name="sumsqs")
        st_a = const_pool.tile([P, NHF], F32, name="st_a")
        st_b = const_pool.tile([P, NHF], F32, name="st_b")
        st_c = const_pool.tile([P, NHF], F32, name="st_c")
        mean_ch = const_pool.tile([P, NHF], F32, name="mean_ch")
        mean_bf = const_pool.tile([P, NHF], BF16, name="mean_bf")
        mqt = const_pool.tile([P, NHF, 2, 1], FP8, name="mqt")
        scS = const_pool.tile([P, NHF], F32, name="scS")
        bias_ch = const_pool.tile([P, NHF], F32, name="bias_ch")
        biasp_ch = const_pool.tile([P, NHF], F32, name="biasp_ch")

        # group-layout weights (partition = group)
        dwg = const_pool.tile([P, 2, 2, 9], F32, name="dwg")    # [g, i, j, t]
        pwg = const_pool.tile([P, 2, 2], F32, name="pwg")       # [g, o, i]
        weffg = const_pool.tile([P, 2, 2, 9], F32, name="weffg")  # [g, o, j, t]

        # dense block-diag weights
        weffd = const_pool.tile([P, NHF, 9, P], F32, name="weffd")   # unscaled
        wtmp = const_pool.tile([P, NHF, 9, P], F32, name="wtmp")     # S*scaled
        wf8 = const_pool.tile([P, NHF, NPAIR, 2, P], FP8, name="wf8")
        wb16 = const_pool.tile([P, NHF, NB16, P], BF16, name="wb16")

        # ACT LUT warm (sqrt/square/identity) off the critical chains
        zz = const_pool.tile([P, 1], F32, name="zz")
        zz2 = const_pool.tile([P, 1], F32, name="zz2")
        with tc.high_priority():
            nc.vector.memset(zz[:], 0.0)
            nc.scalar.activation(out=zz2[:], in_=zz[:], func=ACTF.Square)
            nc.scalar.sqrt(zz2[:], zz[:])
            nc.scalar.activation(
                out=zz2[:], in_=zz[:], func=ACTF.Identity, bias=zz[:], scale=0.0
            )

        # ------------- early DMAs (no stats dependency) -------------
        nc.sync.dma_start(
            out=dwg[:],
            in_=bass.AP(tensor=dw_ext, offset=0, ap=[[36, P], [18, 2], [9, 2], [1, 9]]),
        )
        nc.sync.dma_start(
            out=pwg[:],
            in_=bass.AP(tensor=pw_ext, offset=0, ap=[[4, P], [2, 2], [1, 2]]),
        )

        # ------------- w_eff (group layout) + scatter + load -------------
        with tc.high_priority():
            for o in range(2):
                nc.vector.tensor_scalar(
                    out=weffg[:, o],
                    in0=dwg[:, 0],
                    scalar1=pwg[:, o, 0:1],
                    scalar2=None,
                    op0=OP.mult,
                )
                nc.vector.scalar_tensor_tensor(
                    out=weffg[:, o],
                    in0=dwg[:, 1],
                    scalar=pwg[:, o, 1:2],
                    in1=weffg[:, o],
                    op0=OP.mult,
                    op1=OP.add,
                )

        def emit_scatter_load(hf, scatter_eng):
            # scatter: dst (ci=2a+j, hf, t, co=2a+o) <- weffg[64*hf + a, o, j, t]
            for t in range(9):
                for j in range(2):
                    scatter_eng.dma_start(
                        out=bass.AP(
                            tensor=weff_dram,
                            offset=j * CI_STRIDE + hf * 9 * P + t * P,
                            ap=[[2 * CI_STRIDE + 2, 64], [1, 2]],
                        ),
                        in_=weffg[64 * hf : 64 * (hf + 1), :, j, t],
                    )
            # dense load back: weffd[ci, hf, t, co]
            return nc.sync.dma_start(
                out=weffd[:, hf],
                in_=bass.AP(
                    tensor=weff_dram,
                    offset=hf * 9 * P,
                    ap=[[CI_STRIDE, P], [P, 9], [1, P]],
                ),
            )

        # ------------- x input chunks (xb first, xq rides the tail) -------------
        def ring(hf, idx):
            if hf == 1:
                return nc.sync
            return (nc.sync, nc.scalar, nc.gpsimd)[idx % 3]

        def emit_xb_chunk(hf, ck):
            r0 = ck * CHUNK_TR
            return ring(hf, ck).dma_start(
                out=xbt[hf][:, r0 : r0 + CHUNK_TR, :],
                in_=bass.AP(
                    tensor=xb_ext,
                    offset=hf * P * HP * HP + r0 * HP,
                    ap=[[HP * HP, P], [1, CHUNK_TR * HP]],
                ),
            )

        def emit_xq_chunk(hf, ck):
            r0 = ck * CHUNK_TR
            return ring(hf, ck + 2).dma_start(
                out=xqt[hf][:, r0 : r0 + CHUNK_TR, :],
                in_=bass.AP(
                    tensor=xq_ext,
                    offset=hf * P * HP * HP + r0 * HP,
                    ap=[[HP * HP, P], [1, CHUNK_TR * HP]],
                ),
            )

        with tc.high_priority():
            for ck in range(NCHUNK):
                emit_xb_chunk(0, ck)
            load0_inst = emit_scatter_load(0, nc.scalar)
            for ck in range(NCHUNK):
                emit_xq_chunk(0, ck)
            nc.sync.dma_start(
                out=bias_ch[:],
                in_=bass.AP(tensor=b_ext, offset=0, ap=[[1, P], [P, NHF]]),
            )

        # ------------- per-half pipeline -------------
        h0_last_dve = None
        for hf in range(NHF):
            if hf == 1:
                for ck in range(NCHUNK):
                    inst = emit_xb_chunk(1, ck)
                    if ck == 0:
                        bass._add_dep_helper(
                            inst.ins,
                            load0_inst.ins,
                            sync=True,
                            reason="h1 x stream waits for h0 weight load",
                        )
                for ck in range(NCHUNK):
                    emit_xq_chunk(1, ck)
                emit_scatter_load(1, nc.sync)

            # --- per-chunk stats: sums (DVE), sumsq (ACT), both from xb ---
            for ck in range(NCHUNK):
                r0 = max(1, ck * CHUNK_TR)
                r1 = min(1 + H, (ck + 1) * CHUNK_TR)
                gtr = sq_pool.tile([P, CHUNK_TR, W], BF16, name="gtr")
                ts_inst = nc.vector.tensor_scalar(
                    out=gtr[:, 0 : r1 - r0, :],
                    in0=xbt[hf][:, r0:r1, 1 : 1 + W],
                    scalar1=1.0,
                    scalar2=None,
                    op0=OP.mult,
                    op1=OP.add,
                    accum_out=sums[:, hf, ck : ck + 1],
                )
                if hf == 1 and ck == 0 and h0_last_dve is not None:
                    bass._add_dep_helper(
                        ts_inst.ins,
                        h0_last_dve.ins,
                        sync=True,
                        reason="keep h1 DVE stats behind h0 weight quantize",
                    )
                sq = sq_pool.tile([P, CHUNK_TR, W], BF16, name="sq")
                nc.scalar.activation(
                    out=sq[:, 0 : r1 - r0, :],
                    in_=xbt[hf][:, r0:r1, 1 : 1 + W],
                    func=ACTF.Square,
                    accum_out=sumsqs[:, hf, ck : ck + 1],
                )

            # --- stats finalize ---
            with tc.high_priority():
                nc.vector.tensor_reduce(
                    out=st_a[:, hf : hf + 1], in_=sums[:, hf, :], axis=AX.X, op=OP.add
                )
                nc.vector.tensor_scalar(
                    out=mean_ch[:, hf : hf + 1],
                    in0=st_a[:, hf : hf + 1],
                    scalar1=1.0 / NPIX,
                    scalar2=None,
                    op0=OP.mult,
                )
                nc.vector.tensor_reduce(
                    out=st_b[:, hf : hf + 1], in_=sumsqs[:, hf, :], axis=AX.X, op=OP.add
                )
                nc.vector.tensor_tensor(
                    out=st_c[:, hf : hf + 1],
                    in0=mean_ch[:, hf : hf + 1],
                    in1=mean_ch[:, hf : hf + 1],
                    op=OP.mult,
                )
                nc.vector.scalar_tensor_tensor(
                    out=st_b[:, hf : hf + 1],
                    in0=st_c[:, hf : hf + 1],
                    scalar=float(-NPIX),
                    in1=st_b[:, hf : hf + 1],
                    op0=OP.mult,
                    op1=OP.add,
                )
                nc.vector.tensor_scalar(
                    out=st_b[:, hf : hf + 1],
                    in0=st_b[:, hf : hf + 1],
                    scalar1=1.0 / (NPIX - 1),
                    scalar2=None,
                    op0=OP.mult,
                )
                nc.scalar.sqrt(st_b[:, hf : hf + 1], st_b[:, hf : hf + 1])
                # (std + EPS) / S, then reciprocal -> S/(std+EPS)
                nc.vector.tensor_scalar(
                    out=st_b[:, hf : hf + 1],
                    in0=st_b[:, hf : hf + 1],
                    scalar1=EPS,
                    scalar2=1.0 / S,
                    op0=OP.add,
                    op1=OP.mult,
                )
                nc.vector.reciprocal(scS[:, hf : hf + 1], st_b[:, hf : hf + 1])
                nc.vector.tensor_copy(mean_bf[:, hf : hf + 1], mean_ch[:, hf : hf + 1])
                nc.vector.tensor_copy(mqt[:, hf, 0], mean_ch[:, hf : hf + 1])
                nc.vector.tensor_copy(mqt[:, hf, 1], mean_ch[:, hf : hf + 1])

                # --- scale + quantize the dense weights ---
                nc.vector.tensor_scalar(
                    out=wtmp[:, hf],
                    in0=weffd[:, hf],
                    scalar1=scS[:, hf : hf + 1],
                    scalar2=None,
                    op0=OP.mult,
                )
                # fp8 taps 0..5 -> wf8[hf] ([P, NPAIR*2, P] contiguous)
                nc.vector.tensor_copy(
                    bass.AP(
                        tensor=wf8[:].tensor,
                        offset=hf * NPAIR * 2 * P,
                        ap=[[NHF * NPAIR * 2 * P, P], [P, NPAIR * 2], [1, P]],
                    ),
                    wtmp[:, hf, 0 : 2 * NPAIR, :],
                )
                # bf16 taps 6..8
                h0_last_dve = nc.vector.tensor_copy(
                    wb16[:, hf], wtmp[:, hf, 2 * NPAIR : 9, :]
                )

            # --- border fills (bias = mean, scale = 0) ---
            bias_ap = mean_ch[:, hf : hf + 1]
            with tc.high_priority():
                for tgt in (xbt[hf], xqt[hf]):
                    nc.scalar.activation(
                        out=tgt[:, 1 : 1 + H, 0],
                        in_=tgt[:, 1 : 1 + H, 1],
                        func=ACTF.Identity, bias=bias_ap, scale=0.0,
                    )
                    nc.scalar.activation(
                        out=tgt[:, 1 : 1 + H, HP - 1],
                        in_=tgt[:, 1 : 1 + H, 1],
                        func=ACTF.Identity, bias=bias_ap, scale=0.0,
                    )
                    nc.scalar.activation(
                        out=tgt[:, 0, :],
                        in_=tgt[:, 1, :],
                        func=ACTF.Identity, bias=bias_ap, scale=0.0,
                    )
                    nc.scalar.activation(
                        out=tgt[:, HP - 1, :],
                        in_=tgt[:, 1, :],
                        func=ACTF.Identity, bias=bias_ap, scale=0.0,
                    )

            # --- bias' = bias - (W_s @ mean)/S  (6 accumulated N=1 matmuls) ---
            bps = psum_pool.tile([P, 1], F32, name="bps", tag="ps", bufs=8)
            si = 0
            for p in range(NPAIR):
                nc.tensor.matmul(
                    bps[:],
                    lhsT=wf8[:, hf, p],
                    rhs=mqt[:, hf],
                    start=(si == 0),
                    stop=(si == NSLOT - 1),
                    perf_mode=DR,
                )
                si += 1
            for i in range(NB16):
                nc.tensor.matmul(
                    bps[:],
                    lhsT=wb16[:, hf, i],
                    rhs=mean_bf[:, hf : hf + 1],
                    start=(si == 0),
                    stop=(si == NSLOT - 1),
                )
                si += 1
            nc.vector.scalar_tensor_tensor(
                out=biasp_ch[:, hf : hf + 1],
                in0=bps[:],
                scalar=-1.0 / S,
                in1=bias_ch[:, hf : hf + 1],
                op0=OP.mult,
                op1=OP.add,
            )

            # --- conv: per superblock, 6 slot-instructions x 4 psum tiles ---
            XPITCH = HP * HP  # xqt partition pitch (elements)
            for sb in range(NSB):
                ps = [
                    psum_pool.tile([P, ROWS_PER_MM, W], F32, name="ps", tag="ps", bufs=8)
                    for _ in range(SB_TILES)
                ]
                si = 0
                for p, (t0, t1) in enumerate(FP8_PAIRS):
                    dy0, dx0 = TAPS[t0]
                    dy1, dx1 = TAPS[t1]
                    delta = (dy1 - dy0) * HP + (dx1 - dx0)
                    for k in range(SB_TILES):
                        h0 = sb * SB_ROWS + k * ROWS_PER_MM
                        rhs = bass.AP(
                            tensor=xqt[hf][:].tensor,
                            offset=(h0 + dy0) * HP + dx0,
                            ap=[[XPITCH, P], [delta, 2], [HP, ROWS_PER_MM], [1, W]],
                        )
                        nc.tensor.matmul(
                            ps[k][:],
                            lhsT=wf8[:, hf, p],
                            rhs=rhs,
                            start=(si == 0),
                            stop=(si == NSLOT - 1),
                            perf_mode=DR,
                        )
                    si += 1
                for i, t in enumerate(BF16_TAPS):
                    dy, dx = TAPS[t]
                    for k in range(SB_TILES):
                        h0 = sb * SB_ROWS + k * ROWS_PER_MM
                        nc.tensor.matmul(
                            ps[k][:],
                            lhsT=wb16[:, hf, i],
                            rhs=xbt[hf][:, h0 + dy : h0 + dy + ROWS_PER_MM, dx : dx + W],
                            start=(si == 0),
                            stop=(si == NSLOT - 1),
                        )
                    si += 1
                # epilogue + store in 8-row blocks (2 psum tiles each);
                # alternate ACT/DVE so neither engine bottlenecks
                for half_blk in range(2):
                    stg = stage_pool.tile([P, SB_ROWS // 2, W], BF16, name="stg")
                    for kk in range(2):
                        k = half_blk * 2 + kk
                        dst = stg[:, kk * ROWS_PER_MM : (kk + 1) * ROWS_PER_MM, :]
                        if half_blk == 0:
                            nc.scalar.activation(
                                out=dst,
                                in_=ps[k][:],
                                func=ACTF.Identity,
                                bias=biasp_ch[:, hf : hf + 1],
                                scale=1.0 / S,
                            )
                        else:
                            nc.vector.tensor_scalar(
                                out=dst,
                                in0=ps[k][:],
                                scalar1=1.0 / S,
                                scalar2=biasp_ch[:, hf : hf + 1],
                                op0=OP.mult,
                                op1=OP.add,
                            )
                    nc.sync.dma_start(
                        out=out_ext[
                            hf * P : (hf + 1) * P,
                            sb * SB_ROWS
                            + half_blk * (SB_ROWS // 2) : sb * SB_ROWS
                            + (half_blk + 1) * (SB_ROWS // 2),
                            :,
                        ],
                        in_=stg[:],
                    )

    nc.compile()
    return nc


def get_nc():
    if "nc" not in _CACHED:
        _CACHED["nc"] = build_nc()
    return _CACHED["nc"]


def make_in_maps(x, dw_kernels, pw_kernels, biases):
    x = np.asarray(x, dtype=np.float32)
    dw_kernels = np.asarray(dw_kernels, dtype=np.float32)
    pw_kernels = np.asarray(pw_kernels, dtype=np.float32)
    biases = np.asarray(biases, dtype=np.float32)
    B = x.shape[0]
    in_maps = []
    for i in range(B):
        xq = np.zeros((C, HP, HP), dtype=ml_dtypes.float8_e4m3)
        xb = np.zeros((C, HP, HP), dtype=ml_dtypes.bfloat16)
        xq[:, 1 : 1 + H, 1 : 1 + W] = x[i].astype(ml_dtypes.float8_e4m3)
        xb[:, 1 : 1 + H, 1 : 1 + W] = x[i].astype(ml_dtypes.bfloat16)
        in_maps.append(
            {
                "xq": xq,
                "xb": xb,
                "dw_kernels": np.ascontiguousarray(dw_kernels[i]),
                "pw_kernels": np.ascontiguousarray(pw_kernels[i]),
                "biases": np.ascontiguousarray(biases[i]),
            }
        )
    return in_maps


def postprocess(res, B):
    return np.stack(
        [np.asarray(res.results[i]["out"]).astype(np.float32) for i in range(B)], axis=0
    )


def kernel(x, dw_kernels, pw_kernels, biases):
    B = np.asarray(x).shape[0]
    assert B == 8
    nc = get_nc()
    in_maps = make_in_maps(x, dw_kernels, pw_kernels, biases)
    res = run_bass_kernel_spmd(nc, in_maps, core_ids=list(range(B)))
    return postprocess(res, B)


# revision 15
# speedup vs baseline: 1.3459x; 1.1753x over previous
"""AdaConv2D Trainium2 Bass kernel (fp8-DoubleRow + bf16 hybrid conv).

Problem (per sample): instance-norm(x) -> grouped 3x3 conv (128 groups,
2ch/group, per-sample weights) -> grouped 1x1 conv -> +bias.
B=8, Cin=Cout=256, H=W=128.  Pure data-parallel: 1 sample per NeuronCore.

Math: the 1x1 conv folds into the 3x3 taps (w_eff), the instance norm
folds into the weights (scale per in-channel ci) and bias:
    out = W_s @ x_pad + bias',   W_s[ci,t,co] = w_eff * S/(std_ci+eps)
    bias'[co] = bias[co] - (sum_{ci,t} W_s * mean_ci)/S
with x_pad borders held at mean_ci so border windows cancel, and a
global S=128 pre-scale so fp8-quantized weights stay in e4m3's normal
range (the epilogue multiplies by 1/S).

Precision/speed plan (validated vs f64 reference, ~1.5% L2 global,
gate is 2e-2):
  - Host sends x twice, pre-padded to 130x130: xq = fp8 e4m3 (4.3 MiB)
    and xb = bf16 (8.7 MiB).
  - taps 0..5 run as 3 fp8 DoubleRow matmuls on xq (2 taps per
    instruction; DR costs the same per instruction as one bf16 matmul
    but does 2 taps).
  - taps 6..8 run as bf16 matmuls on xb (near-full precision).
  -> 6 PE instructions per psum tile instead of 9 (bf16-only).
  - Output is written bf16 (8 MiB) and upcast to f32 on the host.

Per-core dataflow:
  - xb streams first (10 chunks/half): DVE accumulates sums, ACT
    accumulates sum-of-squares; the xq stream rides the DMA tail.
  - w_eff scatters (via a zero DRAM scratch) into dense block-diag
    [ci, tap, co] layout, loaded back before stats land; after stats a
    DVE pass scales by S/std and quantizes to the fp8/bf16 lhsT tiles.
  - bias' comes from 6 accumulated N=1 matmuls against the fp8/bf16
    mean, mirroring the conv arithmetic exactly (border cancellation).
  - conv: per 16-row superblock, 4 psum tiles x 6 slot-instructions;
    epilogues alternate ACT/DVE (1/S scale + bias'), emit bf16, DMA out.
"""

import sys

sys.path.insert(0, "/opt/trn_rl_repo")

from contextlib import ExitStack

import numpy as np
import ml_dtypes

from concourse import bacc, bass, mybir, tile
from concourse.bass_utils import run_bass_kernel_spmd

F32 = mybir.dt.float32
BF16 = mybir.dt.bfloat16
FP8 = mybir.dt.float8e4
AX = mybir.AxisListType
OP = mybir.AluOpType
ACTF = mybir.ActivationFunctionType
DR = mybir.MatmulPerfMode.DoubleRow

C = 256          # channels (per sample)
H = W = 128      # spatial
P = 128          # partitions
HP = H + 2       # padded rows/cols (130)
NHF = 2          # channel halves
NCHUNK = 10      # input DMA chunks per half (13 padded rows each)
CHUNK_TR = HP // NCHUNK           # 13 tile rows per chunk
ROWS_PER_MM = 4                   # output rows per psum tile (4*128=512)
SB_TILES = 4                      # psum tiles per superblock
SB_ROWS = ROWS_PER_MM * SB_TILES  # 16 rows per superblock
NSB = H // SB_ROWS                # 8 superblocks per half
NPIX = H * W
EPS = 1e-7
S = 128.0        # weight pre-scale (fp8 range), undone in the epilogue

TAPS = [(t // 3, t % 3) for t in range(9)]
FP8_PAIRS = [(0, 1), (2, 3), (4, 5)]  # DoubleRow tap pairs (xq)
BF16_TAPS = [6, 7, 8]                 # bf16 taps (xb)
NPAIR = len(FP8_PAIRS)
NB16 = len(BF16_TAPS)
NSLOT = NPAIR + NB16

_CACHED = {}


def build_nc():
    nc = bacc.Bacc(trn_type="TRN2")

    xq_ext = nc.declare_dram_parameter("xq", [C, HP, HP], FP8, isOutput=False)
    xb_ext = nc.declare_dram_parameter("xb", [C, HP, HP], BF16, isOutput=False)
    dw_ext = nc.declare_dram_parameter("dw_kernels", [C, 2, 3, 3], F32, isOutput=False)
    pw_ext = nc.declare_dram_parameter("pw_kernels", [C, 2, 1, 1], F32, isOutput=False)
    b_ext = nc.declare_dram_parameter("biases", [C], F32, isOutput=False)
    out_ext = nc.declare_dram_parameter("out", [C, H, W], BF16, isOutput=True)

    # zero-initialized DRAM scratch for the dense block-diag w_eff
    # (runtime scatter only writes the fixed nonzero slots -> idempotent).
    # layout [ci, hf, tap, co] f32
    weff_dram = nc.inline_tensor(
        np.zeros((P, NHF, 9, P), dtype=np.float32), name="weff_zero"
    )
    CI_STRIDE = NHF * 9 * P  # 2304 elements per ci row

    with tile.TileContext(nc) as tc, ExitStack() as ctx:
        const_pool = ctx.enter_context(tc.tile_pool(name="const", bufs=1))
        sq_pool = ctx.enter_context(tc.tile_pool(name="sq", bufs=4))
        psum_pool = ctx.enter_context(tc.tile_pool(name="psum", bufs=8, space="PSUM"))
        stage_pool = ctx.enter_context(tc.tile_pool(name="stage", bufs=6))

        # ---------------- persistent tiles ----------------
        xqt = [const_pool.tile([P, HP, HP], FP8, name=f"xqt{hf}") for hf in range(NHF)]
        xbt = [const_pool.tile([P, HP, HP], BF16, name=f"xbt{hf}") for hf in range(NHF)]

        sums = const_pool.tile([P, NHF, NCHUNK], F32, name="sums")
        sumsqs = const_pool.tile([P, NHF, NCHUNK], F32, name="sumsqs")
        st_a = const_pool.tile([P, NHF], F32, name="st_a")
        st_b = const_pool.tile([P, NHF], F32, name="st_b")
        st_c = const_pool.tile([P, NHF], F32, name="st_c")
        mean_ch = const_pool.tile([P, NHF], F32, name="mean_ch")
        mean_bf = const_pool.tile([P, NHF], BF16, name="mean_bf")
        mqt = const_pool.tile([P, NHF, 2, 1], FP8, name="mqt")
        scS = const_pool.tile([P, NHF], F32, name="scS")
        bias_ch = const_pool.tile([P, NHF], F32, name="bias_ch")
        biasp_ch = const_pool.tile([P, NHF], F32, name="biasp_ch")

        # group-layout weights (partition = group)
        dwg = const_pool.tile([P, 2, 2, 9], F32, name="dwg")    # [g, i, j, t]
        pwg = const_pool.tile([P, 2, 2], F32, name="pwg")       # [g, o, i]
        weffg = const_pool.tile([P, 2, 2, 9], F32, name="weffg")  # [g, o, j, t]

        # dense block-diag weights
        weffd = const_pool.tile([P, NHF, 9, P], F32, name="weffd")   # unscaled
        wtmp = const_pool.tile([P, NHF, 9, P], F32, name="wtmp")     # S*scaled
        wf8 = const_pool.tile([P, NHF, NPAIR, 2, P], FP8, name="wf8")
        wb16 = const_pool.tile([P, NHF, NB16, P], BF16, name="wb16")

        # ACT LUT warm (sqrt/square/identity) off the critical chains
        zz = const_pool.tile([P, 1], F32, name="zz")
        zz2 = const_pool.tile([P, 1], F32, name="zz2")
        with tc.high_priority():
            nc.vector.memset(zz[:], 0.0)
            nc.scalar.activation(out=zz2[:], in_=zz[:], func=ACTF.Square)
            nc.scalar.sqrt(zz2[:], zz[:])
            nc.scalar.activation(
                out=zz2[:], in_=zz[:], func=ACTF.Identity, bias=zz[:], scale=0.0
            )

        # ------------- early DMAs (no stats dependency) -------------
        # weight-path DMAs live on the gpsimd ring so they never queue
        # behind the x stream (sync) or ACT compute (scalar)
        nc.gpsimd.dma_start(
            out=dwg[:],
            in_=bass.AP(tensor=dw_ext, offset=0, ap=[[36, P], [18, 2], [9, 2], [1, 9]]),
        )
        nc.gpsimd.dma_start(
            out=pwg[:],
            in_=bass.AP(tensor=pw_ext, offset=0, ap=[[4, P], [2, 2], [1, 2]]),
        )

        # ------------- w_eff (group layout) + scatter + load -------------
        with tc.high_priority():
            for o in range(2):
                nc.vector.tensor_scalar(
                    out=weffg[:, o],
                    in0=dwg[:, 0],
                    scalar1=pwg[:, o, 0:1],
                    scalar2=None,
                    op0=OP.mult,
                )
                nc.vector.scalar_tensor_tensor(
                    out=weffg[:, o],
                    in0=dwg[:, 1],
                    scalar=pwg[:, o, 1:2],
                    in1=weffg[:, o],
                    op0=OP.mult,
                    op1=OP.add,
                )

        def emit_scatter_load(hf, scatter_eng):
            # scatter: dst (ci=2a+j, hf, t, co=2a+o) <- weffg[64*hf + a, o, j, t]
            for t in range(9):
                for j in range(2):
                    scatter_eng.dma_start(
                        out=bass.AP(
                            tensor=weff_dram,
                            offset=j * CI_STRIDE + hf * 9 * P + t * P,
                            ap=[[2 * CI_STRIDE + 2, 64], [1, 2]],
                        ),
                        in_=weffg[64 * hf : 64 * (hf + 1), :, j, t],
                    )
            # dense load back: weffd[ci, hf, t, co]
            return nc.gpsimd.dma_start(
                out=weffd[:, hf],
                in_=bass.AP(
                    tensor=weff_dram,
                    offset=hf * 9 * P,
                    ap=[[CI_STRIDE, P], [P, 9], [1, P]],
                ),
            )

        # ------------- x input chunks (xb first, xq rides the tail) -------------
        def emit_xb_chunk(hf, ck):
            r0 = ck * CHUNK_TR
            return nc.sync.dma_start(
                out=xbt[hf][:, r0 : r0 + CHUNK_TR, :],
                in_=bass.AP(
                    tensor=xb_ext,
                    offset=hf * P * HP * HP + r0 * HP,
                    ap=[[HP * HP, P], [1, CHUNK_TR * HP]],
                ),
            )

        def emit_xq_chunk(hf, ck):
            r0 = ck * CHUNK_TR
            return nc.sync.dma_start(
                out=xqt[hf][:, r0 : r0 + CHUNK_TR, :],
                in_=bass.AP(
                    tensor=xq_ext,
                    offset=hf * P * HP * HP + r0 * HP,
                    ap=[[HP * HP, P], [1, CHUNK_TR * HP]],
                ),
            )

        with tc.high_priority():
            for ck in range(NCHUNK):
                emit_xb_chunk(0, ck)
            emit_scatter_load(0, nc.gpsimd)
            for ck in range(NCHUNK):
                xq0_last = emit_xq_chunk(0, ck)
            nc.sync.dma_start(
                out=bias_ch[:],
                in_=bass.AP(tensor=b_ext, offset=0, ap=[[1, P], [P, NHF]]),
            )

        # ------------- per-half pipeline -------------
        h0_last_dve = None
        for hf in range(NHF):
            if hf == 1:
                for ck in range(NCHUNK):
                    inst = emit_xb_chunk(1, ck)
                    if ck == 0:
                        bass._add_dep_helper(
                            inst.ins,
                            xq0_last.ins,
                            sync=True,
                            reason="h1 x stream waits for h0 x stream",
                        )
                for ck in range(NCHUNK):
                    emit_xq_chunk(1, ck)
                emit_scatter_load(1, nc.gpsimd)

            # --- per-chunk stats: sums (DVE), sumsq (ACT), both from xb ---
            for ck in range(NCHUNK):
                r0 = max(1, ck * CHUNK_TR)
                r1 = min(1 + H, (ck + 1) * CHUNK_TR)
                gtr = sq_pool.tile([P, CHUNK_TR, W], BF16, name="gtr")
                ts_inst = nc.vector.tensor_scalar(
                    out=gtr[:, 0 : r1 - r0, :],
                    in0=xbt[hf][:, r0:r1, 1 : 1 + W],
                    scalar1=1.0,
                    scalar2=None,
                    op0=OP.mult,
                    op1=OP.add,
                    accum_out=sums[:, hf, ck : ck + 1],
                )
                if hf == 1 and ck == 0 and h0_last_dve is not None:
                    bass._add_dep_helper(
                        ts_inst.ins,
                        h0_last_dve.ins,
                        sync=True,
                        reason="keep h1 DVE stats behind h0 weight quantize",
                    )
                sq = sq_pool.tile([P, CHUNK_TR, W], BF16, name="sq")
                nc.scalar.activation(
                    out=sq[:, 0 : r1 - r0, :],
                    in_=xbt[hf][:, r0:r1, 1 : 1 + W],
                    func=ACTF.Square,
                    accum_out=sumsqs[:, hf, ck : ck + 1],
                )

            # --- stats finalize ---
            with tc.high_priority():
                nc.vector.tensor_reduce(
                    out=st_a[:, hf : hf + 1], in_=sums[:, hf, :], axis=AX.X, op=OP.add
                )
                nc.vector.tensor_scalar(
                    out=mean_ch[:, hf : hf + 1],
                    in0=st_a[:, hf : hf + 1],
                    scalar1=1.0 / NPIX,
                    scalar2=None,
                    op0=OP.mult,
                )
                nc.vector.tensor_reduce(
                    out=st_b[:, hf : hf + 1], in_=sumsqs[:, hf, :], axis=AX.X, op=OP.add
                )
                nc.vector.tensor_tensor(
                    out=st_c[:, hf : hf + 1],
                    in0=mean_ch[:, hf : hf + 1],
                    in1=mean_ch[:, hf : hf + 1],
                    op=OP.mult,
                )
                nc.vector.scalar_tensor_tensor(
                    out=st_b[:, hf : hf + 1],
                    in0=st_c[:, hf : hf + 1],
                    scalar=float(-NPIX),
                    in1=st_b[:, hf : hf + 1],
                    op0=OP.mult,
                    op1=OP.add,
                )
                nc.vector.tensor_scalar(
                    out=st_b[:, hf : hf + 1],
                    in0=st_b[:, hf : hf + 1],
                    scalar1=1.0 / (NPIX - 1),
                    scalar2=None,
                    op0=OP.mult,
                )
                nc.scalar.sqrt(st_b[:, hf : hf + 1], st_b[:, hf : hf + 1])
                # (std + EPS) / S, then reciprocal -> S/(std+EPS)
                nc.vector.tensor_scalar(
                    out=st_b[:, hf : hf + 1],
                    in0=st_b[:, hf : hf + 1],
                    scalar1=EPS,
                    scalar2=1.0 / S,
                    op0=OP.add,
                    op1=OP.mult,
                )
                nc.vector.reciprocal(scS[:, hf : hf + 1], st_b[:, hf : hf + 1])
                nc.vector.tensor_copy(mean_bf[:, hf : hf + 1], mean_ch[:, hf : hf + 1])
                nc.vector.tensor_copy(mqt[:, hf, 0], mean_ch[:, hf : hf + 1])
                nc.vector.tensor_copy(mqt[:, hf, 1], mean_ch[:, hf : hf + 1])

                # --- scale + quantize the dense weights ---
                nc.vector.tensor_scalar(
                    out=wtmp[:, hf],
                    in0=weffd[:, hf],
                    scalar1=scS[:, hf : hf + 1],
                    scalar2=None,
                    op0=OP.mult,
                )
                # fp8 taps 0..5 -> wf8[hf] ([P, NPAIR*2, P] contiguous)
                nc.vector.tensor_copy(
                    bass.AP(
                        tensor=wf8[:].tensor,
                        offset=hf * NPAIR * 2 * P,
                        ap=[[NHF * NPAIR * 2 * P, P], [P, NPAIR * 2], [1, P]],
                    ),
                    wtmp[:, hf, 0 : 2 * NPAIR, :],
                )
                # bf16 taps 6..8
                h0_last_dve = nc.vector.tensor_copy(
                    wb16[:, hf], wtmp[:, hf, 2 * NPAIR : 9, :]
                )

            # --- border fills (bias = mean, scale = 0) ---
            bias_ap = mean_ch[:, hf : hf + 1]
            with tc.high_priority():
                for tgt in (xbt[hf], xqt[hf]):
                    nc.scalar.activation(
                        out=tgt[:, 1 : 1 + H, 0],
                        in_=tgt[:, 1 : 1 + H, 1],
                        func=ACTF.Identity, bias=bias_ap, scale=0.0,
                    )
                    nc.scalar.activation(
                        out=tgt[:, 1 : 1 + H, HP - 1],
                        in_=tgt[:, 1 : 1 + H, 1],
                        func=ACTF.Identity, bias=bias_ap, scale=0.0,
                    )
                    nc.scalar.activation(
                        out=tgt[:, 0, :],
                        in_=tgt[:, 1, :],
                        func=ACTF.Identity, bias=bias_ap, scale=0.0,
                    )
                    nc.scalar.activation(
                        out=tgt[:, HP - 1, :],
                        in_=tgt[:, 1, :],
                        func=ACTF.Identity, bias=bias_ap, scale=0.0,
                    )

            # --- bias' = bias - (W_s @ mean)/S  (6 accumulated N=1 matmuls) ---
            bps = psum_pool.tile([P, 1], F32, name="bps", tag="ps", bufs=8)
            si = 0
            for p in range(NPAIR):
                nc.tensor.matmul(
                    bps[:],
                    lhsT=wf8[:, hf, p],
                    rhs=mqt[:, hf],
                    start=(si == 0),
                    stop=(si == NSLOT - 1),
                    perf_mode=DR,
                )
                si += 1
            for i in range(NB16):
                nc.tensor.matmul(
                    bps[:],
                    lhsT=wb16[:, hf, i],
                    rhs=mean_bf[:, hf : hf + 1],
                    start=(si == 0),
                    stop=(si == NSLOT - 1),
                )
                si += 1
            nc.vector.scalar_tensor_tensor(
                out=biasp_ch[:, hf : hf + 1],
                in0=bps[:],
                scalar=-1.0 / S,
                in1=bias_ch[:, hf : hf + 1],
                op0=OP.mult,
                op1=OP.add,
            )

            # --- conv: per superblock, 6 slot-instructions x 4 psum tiles ---
            XPITCH = HP * HP  # xqt partition pitch (elements)
            for sb in range(NSB):
                ps = [
                    psum_pool.tile([P, ROWS_PER_MM, W], F32, name="ps", tag="ps", bufs=8)
                    for _ in range(SB_TILES)
                ]
                si = 0
                for p, (t0, t1) in enumerate(FP8_PAIRS):
                    dy0, dx0 = TAPS[t0]
                    dy1, dx1 = TAPS[t1]
                    delta = (dy1 - dy0) * HP + (dx1 - dx0)
                    for k in range(SB_TILES):
                        h0 = sb * SB_ROWS + k * ROWS_PER_MM
                        rhs = bass.AP(
                            tensor=xqt[hf][:].tensor,
                            offset=(h0 + dy0) * HP + dx0,
                            ap=[[XPITCH, P], [delta, 2], [HP, ROWS_PER_MM], [1, W]],
                        )
                        nc.tensor.matmul(
                            ps[k][:],
                            lhsT=wf8[:, hf, p],
                            rhs=rhs,
                            start=(si == 0),
                            stop=(si == NSLOT - 1),
                            perf_mode=DR,
                        )
                    si += 1
                for i, t in enumerate(BF16_TAPS):
                    dy, dx = TAPS[t]
                    for k in range(SB_TILES):
                        h0 = sb * SB_ROWS + k * ROWS_PER_MM
                        nc.tensor.matmul(
                            ps[k][:],
                            lhsT=wb16[:, hf, i],
                            rhs=xbt[hf][:, h0 + dy : h0 + dy + ROWS_PER_MM, dx : dx + W],
                            start=(si == 0),
                            stop=(si == NSLOT - 1),
                        )
                    si += 1
                # epilogue + store in 8-row blocks (2 psum tiles each);
                # alternate ACT/DVE so neither engine bottlenecks
                for half_blk in range(2):
                    stg = stage_pool.tile([P, SB_ROWS // 2, W], BF16, name="stg")
                    for kk in range(2):
                        k = half_blk * 2 + kk
                        dst = stg[:, kk * ROWS_PER_MM : (kk + 1) * ROWS_PER_MM, :]
                        if half_blk == 0:
                            nc.scalar.activation(
                                out=dst,
                                in_=ps[k][:],
                                func=ACTF.Identity,
                                bias=biasp_ch[:, hf : hf + 1],
                                scale=1.0 / S,
                            )
                        else:
                            nc.vector.tensor_scalar(
                                out=dst,
                                in0=ps[k][:],
                                scalar1=1.0 / S,
                                scalar2=biasp_ch[:, hf : hf + 1],
                                op0=OP.mult,
                                op1=OP.add,
                            )
                    nc.gpsimd.dma_start(
                        out=out_ext[
                            hf * P : (hf + 1) * P,
                            sb * SB_ROWS
                            + half_blk * (SB_ROWS // 2) : sb * SB_ROWS
                            + (half_blk + 1) * (SB_ROWS // 2),
                            :,
                        ],
                        in_=stg[:],
                    )

    nc.compile()
    return nc


def get_nc():
    if "nc" not in _CACHED:
        _CACHED["nc"] = build_nc()
    return _CACHED["nc"]


def make_in_maps(x, dw_kernels, pw_kernels, biases):
    x = np.asarray(x, dtype=np.float32)
    dw_kernels = np.asarray(dw_kernels, dtype=np.float32)
    pw_kernels = np.asarray(pw_kernels, dtype=np.float32)
    biases = np.asarray(biases, dtype=np.float32)
    B = x.shape[0]
    in_maps = []
    for i in range(B):
        xq = np.zeros((C, HP, HP), dtype=ml_dtypes.float8_e4m3)
        xb = np.zeros((C, HP, HP), dtype=ml_dtypes.bfloat16)
        xq[:, 1 : 1 + H, 1 : 1 + W] = x[i].astype(ml_dtypes.float8_e4m3)
        xb[:, 1 : 1 + H, 1 : 1 + W] = x[i].astype(ml_dtypes.bfloat16)
        in_maps.append(
            {
                "xq": xq,
                "xb": xb,
                "dw_kernels": np.ascontiguousarray(dw_kernels[i]),
                "pw_kernels": np.ascontiguousarray(pw_kernels[i]),
                "biases": np.ascontiguousarray(biases[i]),
            }
        )
    return in_maps


def postprocess(res, B):
    return np.stack(
        [np.asarray(res.results[i]["out"]).astype(np.float32) for i in range(B)], axis=0
    )


def kernel(x, dw_kernels, pw_kernels, biases):
    B = np.asarray(x).shape[0]
    assert B == 8
    nc = get_nc()
    in_maps = make_in_maps(x, dw_kernels, pw_kernels, biases)
    res = run_bass_kernel_spmd(nc, in_maps, core_ids=list(range(B)))
    return postprocess(res, B)


# revision 18
# speedup vs baseline: 1.3659x; 1.0149x over previous
"""AdaConv2D Trainium2 Bass kernel (fp8-DoubleRow + bf16 hybrid conv).

Problem (per sample): instance-norm(x) -> grouped 3x3 conv (128 groups,
2ch/group, per-sample weights) -> grouped 1x1 conv -> +bias.
B=8, Cin=Cout=256, H=W=128.  Pure data-parallel: 1 sample per NeuronCore.

Math: the 1x1 conv folds into the 3x3 taps (w_eff), the instance norm
folds into the weights (scale per in-channel ci) and bias:
    out = W_s @ x_pad + bias',   W_s[ci,t,co] = w_eff * S/(std_ci+eps)
    bias'[co] = bias[co] - (sum_{ci,t} W_s * mean_ci)/S
with x_pad borders held at mean_ci so border windows cancel, and a
global S=128 pre-scale so fp8-quantized weights stay in e4m3's normal
range (the epilogue multiplies by 1/S).

Precision/speed plan (validated vs f64 reference, ~1.5% L2 global,
gate is 2e-2):
  - Host sends x twice, pre-padded to 130x130: xq = fp8 e4m3 (4.3 MiB)
    and xb = bf16 (8.7 MiB).
  - taps 0..5 run as 3 fp8 DoubleRow matmuls on xq (2 taps per
    instruction; DR costs the same per instruction as one bf16 matmul
    but does 2 taps).
  - taps 6..8 run as bf16 matmuls on xb (near-full precision).
  -> 6 PE instructions per psum tile instead of 9 (bf16-only).
  - Output is written bf16 (8 MiB) and upcast to f32 on the host.

Per-core dataflow:
  - xb streams first (10 chunks/half): DVE accumulates sums, ACT
    accumulates sum-of-squares; the xq stream rides the DMA tail.
  - w_eff scatters (via a zero DRAM scratch) into dense block-diag
    [ci, tap, co] layout, loaded back before stats land; after stats a
    DVE pass scales by S/std and quantizes to the fp8/bf16 lhsT tiles.
  - bias' comes from 6 accumulated N=1 matmuls against the fp8/bf16
    mean, mirroring the conv arithmetic exactly (border cancellation).
  - conv: per 16-row superblock, 4 psum tiles x 6 slot-instructions;
    epilogues alternate ACT/DVE (1/S scale + bias'), emit bf16, DMA out.
"""

import sys

sys.path.insert(0, "/opt/trn_rl_repo")

from contextlib import ExitStack

import numpy as np
import ml_dtypes

from concourse import bacc, bass, mybir, tile
from concourse.bass_utils import run_bass_kernel_spmd

F32 = mybir.dt.float32
BF16 = mybir.dt.bfloat16
FP8 = mybir.dt.float8e4
AX = mybir.AxisListType
OP = mybir.AluOpType
ACTF = mybir.ActivationFunctionType
DR = mybir.MatmulPerfMode.DoubleRow

C = 256          # channels (per sample)
H = W = 128      # spatial
P = 128          # partitions
HP = H + 2       # padded rows/cols (130)
NHF = 2          # channel halves
NCHUNK = 10      # input DMA chunks per half (13 padded rows each)
CHUNK_TR = HP // NCHUNK           # 13 tile rows per chunk
ROWS_PER_MM = 4                   # output rows per psum tile (4*128=512)
SB_TILES = 4                      # psum tiles per superblock
SB_ROWS = ROWS_PER_MM * SB_TILES  # 16 rows per superblock
NSB = H // SB_ROWS                # 8 superblocks per half
NPIX = H * W
EPS = 1e-7
S = 128.0        # weight pre-scale (fp8 range), undone in the epilogue

TAPS = [(t // 3, t % 3) for t in range(9)]
FP8_PAIRS = [(0, 1), (2, 3), (4, 5)]  # DoubleRow tap pairs (xq)
BF16_TAPS = [6, 7, 8]                 # bf16 taps (xb)
NPAIR = len(FP8_PAIRS)
NB16 = len(BF16_TAPS)
NSLOT = NPAIR + NB16

_CACHED = {}


def build_nc():
    nc = bacc.Bacc(trn_type="TRN2")

    xq_ext = nc.declare_dram_parameter("xq", [C, HP, HP], FP8, isOutput=False)
    xb_ext = nc.declare_dram_parameter("xb", [C, HP, HP], BF16, isOutput=False)
    dw_ext = nc.declare_dram_parameter("dw_kernels", [C, 2, 3, 3], F32, isOutput=False)
    pw_ext = nc.declare_dram_parameter("pw_kernels", [C, 2, 1, 1], F32, isOutput=False)
    b_ext = nc.declare_dram_parameter("biases", [C], F32, isOutput=False)
    out_ext = nc.declare_dram_parameter("out", [C, H, W], BF16, isOutput=True)

    # zero-initialized DRAM scratch for the dense block-diag w_eff
    # (runtime scatter only writes the fixed nonzero slots -> idempotent).
    # layout [ci, hf, tap, co] f32
    weff_dram = nc.inline_tensor(
        np.zeros((P, NHF, 9, P), dtype=np.float32), name="weff_zero"
    )
    CI_STRIDE = NHF * 9 * P  # 2304 elements per ci row

    with tile.TileContext(nc) as tc, ExitStack() as ctx:
        const_pool = ctx.enter_context(tc.tile_pool(name="const", bufs=1))
        sq_pool = ctx.enter_context(tc.tile_pool(name="sq", bufs=4))
        psum_pool = ctx.enter_context(tc.tile_pool(name="psum", bufs=8, space="PSUM"))
        stage_pool = ctx.enter_context(tc.tile_pool(name="stage", bufs=6))

        # ---------------- persistent tiles ----------------
        xqt = [const_pool.tile([P, HP, HP], FP8, name=f"xqt{hf}") for hf in range(NHF)]
        xbt = [const_pool.tile([P, HP, HP], BF16, name=f"xbt{hf}") for hf in range(NHF)]

        sums = const_pool.tile([P, NHF, NCHUNK], F32, name="sums")
        sumsqs = const_pool.tile([P, NHF, NCHUNK], F32, name="sumsqs")
        st_a = const_pool.tile([P, NHF], F32, name="st_a")
        st_b = const_pool.tile([P, NHF], F32, name="st_b")
        st_c = const_pool.tile([P, NHF], F32, name="st_c")
        mean_ch = const_pool.tile([P, NHF], F32, name="mean_ch")
        mean_bf = const_pool.tile([P, NHF], BF16, name="mean_bf")
        mqt = const_pool.tile([P, NHF, 2, 1], FP8, name="mqt")
        scS = const_pool.tile([P, NHF], F32, name="scS")
        bias_ch = const_pool.tile([P, NHF], F32, name="bias_ch")
        biasp_ch = const_pool.tile([P, NHF], F32, name="biasp_ch")

        # group-layout weights (partition = group)
        dwg = const_pool.tile([P, 2, 2, 9], F32, name="dwg")    # [g, i, j, t]
        pwg = const_pool.tile([P, 2, 2], F32, name="pwg")       # [g, o, i]
        weffg = const_pool.tile([P, 2, 2, 9], F32, name="weffg")  # [g, o, j, t]

        # dense block-diag weights
        weffd = const_pool.tile([P, NHF, 9, P], F32, name="weffd")   # unscaled
        wtmp = const_pool.tile([P, NHF, 9, P], F32, name="wtmp")     # S*scaled
        wf8 = const_pool.tile([P, NHF, NPAIR, 2, P], FP8, name="wf8")
        wb16 = const_pool.tile([P, NHF, NB16, P], BF16, name="wb16")

        # ACT LUT warm (sqrt/square/identity) off the critical chains
        zz = const_pool.tile([P, 1], F32, name="zz")
        zz2 = const_pool.tile([P, 1], F32, name="zz2")
        with tc.high_priority():
            nc.vector.memset(zz[:], 0.0)
            nc.scalar.activation(out=zz2[:], in_=zz[:], func=ACTF.Square)
            nc.scalar.sqrt(zz2[:], zz[:])
            nc.scalar.activation(
                out=zz2[:], in_=zz[:], func=ACTF.Identity, bias=zz[:], scale=0.0
            )

        # ------------- early DMAs (no stats dependency) -------------
        # weight-path DMAs live on the gpsimd ring so they never queue
        # behind the x stream (sync) or ACT compute (scalar)
        nc.gpsimd.dma_start(
            out=dwg[:],
            in_=bass.AP(tensor=dw_ext, offset=0, ap=[[36, P], [18, 2], [9, 2], [1, 9]]),
        )
        nc.gpsimd.dma_start(
            out=pwg[:],
            in_=bass.AP(tensor=pw_ext, offset=0, ap=[[4, P], [2, 2], [1, 2]]),
        )

        # ------------- w_eff (group layout) + scatter + load -------------
        with tc.high_priority():
            for o in range(2):
                nc.vector.tensor_scalar(
                    out=weffg[:, o],
                    in0=dwg[:, 0],
                    scalar1=pwg[:, o, 0:1],
                    scalar2=None,
                    op0=OP.mult,
                )
                nc.vector.scalar_tensor_tensor(
                    out=weffg[:, o],
                    in0=dwg[:, 1],
                    scalar=pwg[:, o, 1:2],
                    in1=weffg[:, o],
                    op0=OP.mult,
                    op1=OP.add,
                )

        def emit_scatter_load(hf, scatter_eng):
            # scatter: dst (ci=2a+j, hf, t, co=2a+o) <- weffg[64*hf + a, o, j, t]
            for t in range(9):
                for j in range(2):
                    scatter_eng.dma_start(
                        out=bass.AP(
                            tensor=weff_dram,
                            offset=j * CI_STRIDE + hf * 9 * P + t * P,
                            ap=[[2 * CI_STRIDE + 2, 64], [1, 2]],
                        ),
                        in_=weffg[64 * hf : 64 * (hf + 1), :, j, t],
                    )
            # dense load back: weffd[ci, hf, t, co]
            return nc.gpsimd.dma_start(
                out=weffd[:, hf],
                in_=bass.AP(
                    tensor=weff_dram,
                    offset=hf * 9 * P,
                    ap=[[CI_STRIDE, P], [P, 9], [1, P]],
                ),
            )

        # ------------- x input chunks (xb first, xq rides the tail) -------------
        def emit_xb_chunk(hf, ck):
            r0 = ck * CHUNK_TR
            return nc.sync.dma_start(
                out=xbt[hf][:, r0 : r0 + CHUNK_TR, :],
                in_=bass.AP(
                    tensor=xb_ext,
                    offset=hf * P * HP * HP + r0 * HP,
                    ap=[[HP * HP, P], [1, CHUNK_TR * HP]],
                ),
            )

        def emit_xq_chunk(hf, ck):
            r0 = ck * CHUNK_TR
            return nc.sync.dma_start(
                out=xqt[hf][:, r0 : r0 + CHUNK_TR, :],
                in_=bass.AP(
                    tensor=xq_ext,
                    offset=hf * P * HP * HP + r0 * HP,
                    ap=[[HP * HP, P], [1, CHUNK_TR * HP]],
                ),
            )

        with tc.high_priority():
            for ck in range(NCHUNK):
                emit_xb_chunk(0, ck)
            emit_scatter_load(0, nc.gpsimd)
            for ck in range(NCHUNK):
                xq0_last = emit_xq_chunk(0, ck)
            nc.sync.dma_start(
                out=bias_ch[:],
                in_=bass.AP(tensor=b_ext, offset=0, ap=[[1, P], [P, NHF]]),
            )

        # ------------- per-half pipeline -------------
        h0_last_dve = None
        for hf in range(NHF):
            if hf == 1:
                for ck in range(NCHUNK):
                    inst = emit_xb_chunk(1, ck)
                    if ck == 0:
                        bass._add_dep_helper(
                            inst.ins,
                            xq0_last.ins,
                            sync=True,
                            reason="h1 x stream waits for h0 x stream",
                        )
                for ck in range(NCHUNK):
                    emit_xq_chunk(1, ck)
                emit_scatter_load(1, nc.gpsimd)

            # --- per-chunk stats: sums (DVE), sumsq (ACT), both from xb ---
            for ck in range(NCHUNK):
                r0 = max(1, ck * CHUNK_TR)
                r1 = min(1 + H, (ck + 1) * CHUNK_TR)
                gtr = sq_pool.tile([P, CHUNK_TR, W], BF16, name="gtr")
                ts_inst = nc.vector.tensor_scalar(
                    out=gtr[:, 0 : r1 - r0, :],
                    in0=xbt[hf][:, r0:r1, 1 : 1 + W],
                    scalar1=1.0,
                    scalar2=None,
                    op0=OP.mult,
                    op1=OP.add,
                    accum_out=sums[:, hf, ck : ck + 1],
                )
                if hf == 1 and ck == 0 and h0_last_dve is not None:
                    bass._add_dep_helper(
                        ts_inst.ins,
                        h0_last_dve.ins,
                        sync=True,
                        reason="keep h1 DVE stats behind h0 weight quantize",
                    )
                sq = sq_pool.tile([P, CHUNK_TR, W], BF16, name="sq")
                sq_inst = nc.scalar.activation(
                    out=sq[:, 0 : r1 - r0, :],
                    in_=xbt[hf][:, r0:r1, 1 : 1 + W],
                    func=ACTF.Square,
                    accum_out=sumsqs[:, hf, ck : ck + 1],
                )

            # --- stats finalize ---
            with tc.high_priority():
                nc.vector.tensor_reduce(
                    out=st_a[:, hf : hf + 1], in_=sums[:, hf, :], axis=AX.X, op=OP.add
                )
                nc.vector.tensor_scalar(
                    out=mean_ch[:, hf : hf + 1],
                    in0=st_a[:, hf : hf + 1],
                    scalar1=1.0 / NPIX,
                    scalar2=None,
                    op0=OP.mult,
                )
                nc.vector.tensor_reduce(
                    out=st_b[:, hf : hf + 1], in_=sumsqs[:, hf, :], axis=AX.X, op=OP.add
                )
                nc.vector.tensor_tensor(
                    out=st_c[:, hf : hf + 1],
                    in0=mean_ch[:, hf : hf + 1],
                    in1=mean_ch[:, hf : hf + 1],
                    op=OP.mult,
                )
                nc.vector.scalar_tensor_tensor(
                    out=st_b[:, hf : hf + 1],
                    in0=st_c[:, hf : hf + 1],
                    scalar=float(-NPIX),
                    in1=st_b[:, hf : hf + 1],
                    op0=OP.mult,
                    op1=OP.add,
                )
                nc.vector.tensor_scalar(
                    out=st_b[:, hf : hf + 1],
                    in0=st_b[:, hf : hf + 1],
                    scalar1=1.0 / (NPIX - 1),
                    scalar2=None,
                    op0=OP.mult,
                )
                nc.scalar.sqrt(st_b[:, hf : hf + 1], st_b[:, hf : hf + 1])
                # (std + EPS) / S, then reciprocal -> S/(std+EPS)
                nc.vector.tensor_scalar(
                    out=st_b[:, hf : hf + 1],
                    in0=st_b[:, hf : hf + 1],
                    scalar1=EPS,
                    scalar2=1.0 / S,
                    op0=OP.add,
                    op1=OP.mult,
                )
                nc.vector.reciprocal(scS[:, hf : hf + 1], st_b[:, hf : hf + 1])
                nc.vector.tensor_copy(mean_bf[:, hf : hf + 1], mean_ch[:, hf : hf + 1])
                nc.vector.tensor_copy(mqt[:, hf, 0], mean_ch[:, hf : hf + 1])
                nc.vector.tensor_copy(mqt[:, hf, 1], mean_ch[:, hf : hf + 1])

                # --- scale + quantize the dense weights ---
                nc.vector.tensor_scalar(
                    out=wtmp[:, hf],
                    in0=weffd[:, hf],
                    scalar1=scS[:, hf : hf + 1],
                    scalar2=None,
                    op0=OP.mult,
                )
                # fp8 taps 0..5 -> wf8[hf] ([P, NPAIR*2, P] contiguous)
                nc.vector.tensor_copy(
                    bass.AP(
                        tensor=wf8[:].tensor,
                        offset=hf * NPAIR * 2 * P,
                        ap=[[NHF * NPAIR * 2 * P, P], [P, NPAIR * 2], [1, P]],
                    ),
                    wtmp[:, hf, 0 : 2 * NPAIR, :],
                )
                # bf16 taps 6..8
                h0_last_dve = nc.vector.tensor_copy(
                    wb16[:, hf], wtmp[:, hf, 2 * NPAIR : 9, :]
                )

            # --- border fills (bias = mean, scale = 0); pinned after the
            # half's last square so the scheduler can't hoist them into the
            # middle of the ACT stats stream ---
            bias_ap = mean_ch[:, hf : hf + 1]
            for tgt in (xbt[hf], xqt[hf]):
                edges = [
                    ((slice(None), slice(1, 1 + H), 0), (slice(None), slice(1, 1 + H), 1)),
                    ((slice(None), slice(1, 1 + H), HP - 1), (slice(None), slice(1, 1 + H), 1)),
                    ((slice(None), 0, slice(None)), (slice(None), 1, slice(None))),
                    ((slice(None), HP - 1, slice(None)), (slice(None), 1, slice(None))),
                ]
                for osl, isl in edges:
                    bi = nc.scalar.activation(
                        out=tgt[osl], in_=tgt[isl],
                        func=ACTF.Identity, bias=bias_ap, scale=0.0,
                    )
                    bass._add_dep_helper(
                        bi.ins, sq_inst.ins, sync=True,
                        reason="border fills after the half's ACT stats stream",
                    )

            # --- bias' = bias - (W_s @ mean)/S  (6 accumulated N=1 matmuls) ---
            bps = psum_pool.tile([P, 1], F32, name="bps", tag="ps", bufs=8)
            si = 0
            for p in range(NPAIR):
                nc.tensor.matmul(
                    bps[:],
                    lhsT=wf8[:, hf, p],
                    rhs=mqt[:, hf],
                    start=(si == 0),
                    stop=(si == NSLOT - 1),
                    perf_mode=DR,
                )
                si += 1
            for i in range(NB16):
                nc.tensor.matmul(
                    bps[:],
                    lhsT=wb16[:, hf, i],
                    rhs=mean_bf[:, hf : hf + 1],
                    start=(si == 0),
                    stop=(si == NSLOT - 1),
                )
                si += 1
            nc.vector.scalar_tensor_tensor(
                out=biasp_ch[:, hf : hf + 1],
                in0=bps[:],
                scalar=-1.0 / S,
                in1=bias_ch[:, hf : hf + 1],
                op0=OP.mult,
                op1=OP.add,
            )

            # --- conv: per superblock, 6 slot-instructions x 4 psum tiles ---
            XPITCH = HP * HP  # xqt partition pitch (elements)
            for sb in range(NSB):
                ps = [
                    psum_pool.tile([P, ROWS_PER_MM, W], F32, name="ps", tag="ps", bufs=8)
                    for _ in range(SB_TILES)
                ]
                si = 0
                for p, (t0, t1) in enumerate(FP8_PAIRS):
                    dy0, dx0 = TAPS[t0]
                    dy1, dx1 = TAPS[t1]
                    delta = (dy1 - dy0) * HP + (dx1 - dx0)
                    for k in range(SB_TILES):
                        h0 = sb * SB_ROWS + k * ROWS_PER_MM
                        rhs = bass.AP(
                            tensor=xqt[hf][:].tensor,
                            offset=(h0 + dy0) * HP + dx0,
                            ap=[[XPITCH, P], [delta, 2], [HP, ROWS_PER_MM], [1, W]],
                        )
                        nc.tensor.matmul(
                            ps[k][:],
                            lhsT=wf8[:, hf, p],
                            rhs=rhs,
                            start=(si == 0),
                            stop=(si == NSLOT - 1),
                            perf_mode=DR,
                        )
                    si += 1
                for i, t in enumerate(BF16_TAPS):
                    dy, dx = TAPS[t]
                    for k in range(SB_TILES):
                        h0 = sb * SB_ROWS + k * ROWS_PER_MM
                        nc.tensor.matmul(
                            ps[k][:],
                            lhsT=wb16[:, hf, i],
                            rhs=xbt[hf][:, h0 + dy : h0 + dy + ROWS_PER_MM, dx : dx + W],
                            start=(si == 0),
                            stop=(si == NSLOT - 1),
                        )
                    si += 1
                # epilogue + store in 8-row blocks (2 psum tiles each);
                # alternate ACT/DVE so neither engine bottlenecks
                for half_blk in range(2):
                    stg = stage_pool.tile([P, SB_ROWS // 2, W], BF16, name="stg")
                    for kk in range(2):
                        k = half_blk * 2 + kk
                        dst = stg[:, kk * ROWS_PER_MM : (kk + 1) * ROWS_PER_MM, :]
                        if half_blk == 0:
                            nc.scalar.activation(
                                out=dst,
                                in_=ps[k][:],
                                func=ACTF.Identity,
                                bias=biasp_ch[:, hf : hf + 1],
                                scale=1.0 / S,
                            )
                        else:
                            nc.vector.tensor_scalar(
                                out=dst,
                                in0=ps[k][:],
                                scalar1=1.0 / S,
                                scalar2=biasp_ch[:, hf : hf + 1],
                                op0=OP.mult,
                                op1=OP.add,
                            )
                    out_eng = nc.gpsimd if half_blk == 0 else nc.sync
                    out_eng.dma_start(
                        out=out_ext[
                            hf * P : (hf + 1) * P,
                            sb * SB_ROWS
                            + half_blk * (SB_ROWS // 2) : sb * SB_ROWS
                            + (half_blk + 1) * (SB_ROWS // 2),
                            :,
                        ],
                        in_=stg[:],
                    )

    nc.compile()
    return nc


def get_nc():
    if "nc" not in _CACHED:
        _CACHED["nc"] = build_nc()
    return _CACHED["nc"]


def make_in_maps(x, dw_kernels, pw_kernels, biases):
    x = np.asarray(x, dtype=np.float32)
    dw_kernels = np.asarray(dw_kernels, dtype=np.float32)
    pw_kernels = np.asarray(pw_kernels, dtype=np.float32)
    biases = np.asarray(biases, dtype=np.float32)
    B = x.shape[0]
    in_maps = []
    for i in range(B):
        xq = np.zeros((C, HP, HP), dtype=ml_dtypes.float8_e4m3)
        xb = np.zeros((C, HP, HP), dtype=ml_dtypes.bfloat16)
        xq[:, 1 : 1 + H, 1 : 1 + W] = x[i].astype(ml_dtypes.float8_e4m3)
        xb[:, 1 : 1 + H, 1 : 1 + W] = x[i].astype(ml_dtypes.bfloat16)
        in_maps.append(
            {
                "xq": xq,
                "xb": xb,
                "dw_kernels": np.ascontiguousarray(dw_kernels[i]),
                "pw_kernels": np.ascontiguousarray(pw_kernels[i]),
                "biases": np.ascontiguousarray(biases[i]),
            }
        )
    return in_maps


def postprocess(res, B):
    return np.stack(
        [np.asarray(res.results[i]["out"]).astype(np.float32) for i in range(B)], axis=0
    )


def kernel(x, dw_kernels, pw_kernels, biases):
    B = np.asarray(x).shape[0]
    assert B == 8
    nc = get_nc()
    in_maps = make_in_maps(x, dw_kernels, pw_kernels, biases)
    res = run_bass_kernel_spmd(nc, in_maps, core_ids=list(range(B)))
    return postprocess(res, B)


# revision 25
# speedup vs baseline: 1.4624x; 1.0706x over previous
"""AdaConv2D Trainium2 Bass kernel (fp8-DoubleRow + bf16 hybrid conv).

Problem (per sample): instance-norm(x) -> grouped 3x3 conv (128 groups,
2ch/group, per-sample weights) -> grouped 1x1 conv -> +bias.
B=8, Cin=Cout=256, H=W=128.  Pure data-parallel: 1 sample per NeuronCore.

Math: the 1x1 conv folds into the 3x3 taps (w_eff), the instance norm
folds into the weights (scale per in-channel ci) and bias:
    out = W_s @ x_pad + bias',   W_s[ci,t,co] = w_eff * S/(std_ci+eps)
    bias'[co] = bias[co] - (sum_{ci,t} W_s * mean_ci)/S
with x_pad borders held at mean_ci so border windows cancel, and a
global S=128 pre-scale so fp8-quantized weights stay in e4m3's normal
range (the epilogue multiplies by 1/S).

Precision/speed plan (validated vs f64 reference, ~1.5% L2 global,
gate is 2e-2):
  - Host sends x twice, pre-padded to 130x130: xq = fp8 e4m3 (4.3 MiB)
    and xb = bf16 (8.7 MiB).
  - taps 0..5 run as 3 fp8 DoubleRow matmuls on xq (2 taps per
    instruction; DR costs the same per instruction as one bf16 matmul
    but does 2 taps).
  - taps 6..8 run as bf16 matmuls on xb (near-full precision).
  -> 6 PE instructions per psum tile instead of 9 (bf16-only).
  - Output is written bf16 (8 MiB) and upcast to f32 on the host.

Per-core dataflow:
  - xb streams first (10 chunks/half): DVE accumulates sums, ACT
    accumulates sum-of-squares; the xq stream rides the DMA tail.
  - w_eff scatters (via a zero DRAM scratch) into dense block-diag
    [ci, tap, co] layout, loaded back before stats land; after stats a
    DVE pass scales by S/std and quantizes to the fp8/bf16 lhsT tiles.
  - bias' comes from 6 accumulated N=1 matmuls against the fp8/bf16
    mean, mirroring the conv arithmetic exactly (border cancellation).
  - conv: per 16-row superblock, 4 psum tiles x 6 slot-instructions;
    epilogues alternate ACT/DVE (1/S scale + bias'), emit bf16, DMA out.
"""

import sys

sys.path.insert(0, "/opt/trn_rl_repo")

from contextlib import ExitStack

import numpy as np
import ml_dtypes

from concourse import bacc, bass, mybir, tile
from concourse.bass_utils import run_bass_kernel_spmd

F32 = mybir.dt.float32
BF16 = mybir.dt.bfloat16
FP8 = mybir.dt.float8e4
AX = mybir.AxisListType
OP = mybir.AluOpType
ACTF = mybir.ActivationFunctionType
DR = mybir.MatmulPerfMode.DoubleRow

C = 256          # channels (per sample)
H = W = 128      # spatial
P = 128          # partitions
HP = H + 2       # padded rows/cols (130)
NHF = 2          # channel halves
NCHUNK = 10      # input DMA chunks per half (13 padded rows each)
CHUNK_TR = HP // NCHUNK           # 13 tile rows per chunk
ROWS_PER_MM = 4                   # output rows per psum tile (4*128=512)
SB_TILES = 4                      # psum tiles per superblock
SB_ROWS = ROWS_PER_MM * SB_TILES  # 16 rows per superblock
NSB = H // SB_ROWS                # 8 superblocks per half
NPIX = H * W
EPS = 1e-7
S = 128.0        # weight pre-scale (fp8 range), undone in the epilogue

TAPS = [(t // 3, t % 3) for t in range(9)]
FP8_PAIRS = [(0, 1), (2, 3), (4, 5)]  # DoubleRow tap pairs (xq)
BF16_TAPS = [6, 7, 8]                 # bf16 taps (xb)
NPAIR = len(FP8_PAIRS)
NB16 = len(BF16_TAPS)
NSLOT = NPAIR + NB16

_CACHED = {}


def build_nc():
    nc = bacc.Bacc(trn_type="TRN2")

    xq_ext = nc.declare_dram_parameter("xq", [C, HP, HP], FP8, isOutput=False)
    xb_ext = nc.declare_dram_parameter("xb", [C, HP, HP], BF16, isOutput=False)
    dw_ext = nc.declare_dram_parameter("dw_kernels", [C, 2, 3, 3], F32, isOutput=False)
    pw_ext = nc.declare_dram_parameter("pw_kernels", [C, 2, 1, 1], F32, isOutput=False)
    b_ext = nc.declare_dram_parameter("biases", [C], F32, isOutput=False)
    out_ext = nc.declare_dram_parameter("out", [C, H, W], BF16, isOutput=True)

    # zero-initialized DRAM scratch for the dense block-diag w_eff
    # (runtime scatter only writes the fixed nonzero slots -> idempotent).
    # layout [ci, hf, tap, co] f32
    weff_dram = nc.inline_tensor(
        np.zeros((P, NHF, 9, P), dtype=np.float32), name="weff_zero"
    )
    CI_STRIDE = NHF * 9 * P  # 2304 elements per ci row

    with tile.TileContext(nc) as tc, ExitStack() as ctx:
        const_pool = ctx.enter_context(tc.tile_pool(name="const", bufs=1))
        sq_pool = ctx.enter_context(tc.tile_pool(name="sq", bufs=4))
        psum_pool = ctx.enter_context(tc.tile_pool(name="psum", bufs=8, space="PSUM"))
        stage_pool = ctx.enter_context(tc.tile_pool(name="stage", bufs=6))

        # ---------------- persistent tiles ----------------
        xqt = [const_pool.tile([P, HP, HP], FP8, name=f"xqt{hf}") for hf in range(NHF)]
        xbt = [const_pool.tile([P, HP, HP], BF16, name=f"xbt{hf}") for hf in range(NHF)]

        sums = const_pool.tile([P, NHF, NCHUNK], F32, name="sums")
        sumsqs = const_pool.tile([P, NHF, NCHUNK], F32, name="sumsqs")
        st_a = const_pool.tile([P, NHF], F32, name="st_a")
        st_b = const_pool.tile([P, NHF], F32, name="st_b")
        st_c = const_pool.tile([P, NHF], F32, name="st_c")
        mean_ch = const_pool.tile([P, NHF], F32, name="mean_ch")
        mean_bf = const_pool.tile([P, NHF], BF16, name="mean_bf")
        mqt = const_pool.tile([P, NHF, 2, 1], FP8, name="mqt")
        scS = const_pool.tile([P, NHF], F32, name="scS")
        bias_ch = const_pool.tile([P, NHF], F32, name="bias_ch")
        biasp_ch = const_pool.tile([P, NHF], F32, name="biasp_ch")

        # group-layout weights (partition = group)
        dwg = const_pool.tile([P, 2, 2, 9], F32, name="dwg")    # [g, i, j, t]
        pwg = const_pool.tile([P, 2, 2], F32, name="pwg")       # [g, o, i]
        weffg = const_pool.tile([P, 2, 9, 2], F32, name="weffg")  # [g, j, t, o]

        # dense block-diag weights (per-half tiles so half 0's quantize
        # never picks up a false whole-tile dep on half 1's load)
        weffd = [const_pool.tile([P, 9, P], F32, name=f"weffd{h}") for h in range(NHF)]
        wtmp = [const_pool.tile([P, 9, P], F32, name=f"wtmp{h}") for h in range(NHF)]
        wf8 = const_pool.tile([P, NHF, NPAIR, 2, P], FP8, name="wf8")
        wb16 = const_pool.tile([P, NHF, NB16, P], BF16, name="wb16")

        # ACT LUT warm (sqrt/square/identity) off the critical chains
        zz = const_pool.tile([P, 1], F32, name="zz")
        zz2 = const_pool.tile([P, 1], F32, name="zz2")
        with tc.high_priority():
            nc.vector.memset(zz[:], 0.0)
            nc.scalar.activation(out=zz2[:], in_=zz[:], func=ACTF.Square)
            nc.scalar.sqrt(zz2[:], zz[:])
            nc.scalar.activation(
                out=zz2[:], in_=zz[:], func=ACTF.Identity, bias=zz[:], scale=0.0
            )

        # ------------- early DMAs (no stats dependency) -------------
        # weight-path DMAs live on the gpsimd ring so they never queue
        # behind the x stream (sync) or ACT compute (scalar)
        nc.gpsimd.dma_start(
            out=dwg[:],
            in_=bass.AP(tensor=dw_ext, offset=0, ap=[[36, P], [18, 2], [9, 2], [1, 9]]),
        )
        nc.gpsimd.dma_start(
            out=pwg[:],
            in_=bass.AP(tensor=pw_ext, offset=0, ap=[[4, P], [2, 2], [1, 2]]),
        )

        # ------------- w_eff (group layout) + scatter + load -------------
        with tc.high_priority():
            for o in range(2):
                nc.vector.tensor_scalar(
                    out=weffg[:, :, :, o],
                    in0=dwg[:, 0],
                    scalar1=pwg[:, o, 0:1],
                    scalar2=None,
                    op0=OP.mult,
                )
                nc.vector.scalar_tensor_tensor(
                    out=weffg[:, :, :, o],
                    in0=dwg[:, 1],
                    scalar=pwg[:, o, 1:2],
                    in1=weffg[:, :, :, o],
                    op0=OP.mult,
                    op1=OP.add,
                )

        def emit_scatter_load(hf, scatter_eng):
            # scatter: dst (ci=2a+j, hf, t, co=2a+o) <- weffg[64*hf + a, o, j, t]
            # one 3-dim DMA per j: dst dims (a, t, o), src dims (a, t, o)
            for j in range(2):
                scatter_eng.dma_start(
                    out=bass.AP(
                        tensor=weff_dram,
                        offset=j * CI_STRIDE + hf * 9 * P,
                        ap=[[2 * CI_STRIDE + 2, 64], [P, 9], [1, 2]],
                    ),
                    in_=bass.AP(
                        tensor=weffg[:].tensor,
                        offset=(64 * hf) * 36 + j * 18,
                        ap=[[36, 64], [2, 9], [1, 2]],
                    ),
                )
            # dense load back: weffd[hf][ci, t, co]
            return nc.gpsimd.dma_start(
                out=weffd[hf][:],
                in_=bass.AP(
                    tensor=weff_dram,
                    offset=hf * 9 * P,
                    ap=[[CI_STRIDE, P], [P, 9], [1, P]],
                ),
            )

        # ------------- x input chunks (xb first, xq rides the tail) -------------
        def emit_xb_chunk(hf, ck):
            r0 = ck * CHUNK_TR
            return nc.sync.dma_start(
                out=xbt[hf][:, r0 : r0 + CHUNK_TR, :],
                in_=bass.AP(
                    tensor=xb_ext,
                    offset=hf * P * HP * HP + r0 * HP,
                    ap=[[HP * HP, P], [1, CHUNK_TR * HP]],
                ),
            )

        def emit_xq_chunk(hf, ck):
            r0 = ck * CHUNK_TR
            return nc.sync.dma_start(
                out=xqt[hf][:, r0 : r0 + CHUNK_TR, :],
                in_=bass.AP(
                    tensor=xq_ext,
                    offset=hf * P * HP * HP + r0 * HP,
                    ap=[[HP * HP, P], [1, CHUNK_TR * HP]],
                ),
            )

        with tc.high_priority():
            for ck in range(NCHUNK):
                emit_xb_chunk(0, ck)
            emit_scatter_load(0, nc.gpsimd)
            for ck in range(NCHUNK):
                xq0_last = emit_xq_chunk(0, ck)
            nc.sync.dma_start(
                out=bias_ch[:],
                in_=bass.AP(tensor=b_ext, offset=0, ap=[[1, P], [P, NHF]]),
            )

        # ------------- per-half pipeline -------------
        h0_last_dve = None
        for hf in range(NHF):
            if hf == 1:
                for ck in range(NCHUNK):
                    inst = emit_xb_chunk(1, ck)
                    if ck == 0:
                        bass._add_dep_helper(
                            inst.ins,
                            xq0_last.ins,
                            sync=True,
                            reason="h1 x stream waits for h0 x stream",
                        )
                for ck in range(NCHUNK):
                    emit_xq_chunk(1, ck)
                emit_scatter_load(1, nc.gpsimd)

            # --- per-chunk stats: sums (DVE), sumsq (ACT), both from xb ---
            for ck in range(NCHUNK):
                r0 = max(1, ck * CHUNK_TR)
                r1 = min(1 + H, (ck + 1) * CHUNK_TR)
                gtr = sq_pool.tile([P, CHUNK_TR, W], BF16, name="gtr")
                ts_inst = nc.vector.tensor_scalar(
                    out=gtr[:, 0 : r1 - r0, :],
                    in0=xbt[hf][:, r0:r1, 1 : 1 + W],
                    scalar1=1.0,
                    scalar2=None,
                    op0=OP.mult,
                    op1=OP.add,
                    accum_out=sums[:, hf, ck : ck + 1],
                )
                if hf == 1 and ck == 0 and h0_last_dve is not None:
                    bass._add_dep_helper(
                        ts_inst.ins,
                        h0_last_dve.ins,
                        sync=True,
                        reason="keep h1 DVE stats behind h0 weight quantize",
                    )
                sq = sq_pool.tile([P, CHUNK_TR, W], BF16, name="sq")
                sq_inst = nc.scalar.activation(
                    out=sq[:, 0 : r1 - r0, :],
                    in_=xbt[hf][:, r0:r1, 1 : 1 + W],
                    func=ACTF.Square,
                    accum_out=sumsqs[:, hf, ck : ck + 1],
                )

            # --- stats finalize ---
            with tc.high_priority():
                nc.vector.tensor_reduce(
                    out=st_a[:, hf : hf + 1], in_=sums[:, hf, :], axis=AX.X, op=OP.add
                )
                nc.vector.tensor_scalar(
                    out=mean_ch[:, hf : hf + 1],
                    in0=st_a[:, hf : hf + 1],
                    scalar1=1.0 / NPIX,
                    scalar2=None,
                    op0=OP.mult,
                )
                nc.vector.tensor_reduce(
                    out=st_b[:, hf : hf + 1], in_=sumsqs[:, hf, :], axis=AX.X, op=OP.add
                )
                nc.vector.tensor_tensor(
                    out=st_c[:, hf : hf + 1],
                    in0=mean_ch[:, hf : hf + 1],
                    in1=mean_ch[:, hf : hf + 1],
                    op=OP.mult,
                )
                nc.vector.scalar_tensor_tensor(
                    out=st_b[:, hf : hf + 1],
                    in0=st_c[:, hf : hf + 1],
                    scalar=float(-NPIX),
                    in1=st_b[:, hf : hf + 1],
                    op0=OP.mult,
                    op1=OP.add,
                )
                nc.vector.tensor_scalar(
                    out=st_b[:, hf : hf + 1],
                    in0=st_b[:, hf : hf + 1],
                    scalar1=1.0 / (NPIX - 1),
                    scalar2=None,
                    op0=OP.mult,
                )
                nc.scalar.sqrt(st_b[:, hf : hf + 1], st_b[:, hf : hf + 1])
                # (std + EPS) / S, then reciprocal -> S/(std+EPS)
                nc.vector.tensor_scalar(
                    out=st_b[:, hf : hf + 1],
                    in0=st_b[:, hf : hf + 1],
                    scalar1=EPS,
                    scalar2=1.0 / S,
                    op0=OP.add,
                    op1=OP.mult,
                )
                nc.vector.reciprocal(scS[:, hf : hf + 1], st_b[:, hf : hf + 1])
                nc.vector.tensor_copy(mean_bf[:, hf : hf + 1], mean_ch[:, hf : hf + 1])
                nc.vector.tensor_copy(mqt[:, hf, 0], mean_ch[:, hf : hf + 1])
                nc.vector.tensor_copy(mqt[:, hf, 1], mean_ch[:, hf : hf + 1])

                # --- scale + quantize the dense weights ---
                nc.vector.tensor_scalar(
                    out=wtmp[hf][:],
                    in0=weffd[hf][:],
                    scalar1=scS[:, hf : hf + 1],
                    scalar2=None,
                    op0=OP.mult,
                )
                # fp8 taps -> wf8[hf] ([P, NPAIR*2, P] contiguous)
                nc.vector.tensor_copy(
                    bass.AP(
                        tensor=wf8[:].tensor,
                        offset=hf * NPAIR * 2 * P,
                        ap=[[NHF * NPAIR * 2 * P, P], [P, NPAIR * 2], [1, P]],
                    ),
                    wtmp[hf][:, 0 : 2 * NPAIR, :],
                )
                # bf16 taps
                h0_last_dve = nc.vector.tensor_copy(
                    wb16[:, hf], wtmp[hf][:, 2 * NPAIR : 9, :]
                )

            # --- border fills (bias = mean, scale = 0); pinned after the
            # half's last square so the scheduler can't hoist them into the
            # middle of the ACT stats stream ---
            bias_ap = mean_ch[:, hf : hf + 1]
            for tgt in (xbt[hf], xqt[hf]):
                edges = [
                    ((slice(None), slice(1, 1 + H), 0), (slice(None), slice(1, 1 + H), 1)),
                    ((slice(None), slice(1, 1 + H), HP - 1), (slice(None), slice(1, 1 + H), 1)),
                    ((slice(None), 0, slice(None)), (slice(None), 1, slice(None))),
                    ((slice(None), HP - 1, slice(None)), (slice(None), 1, slice(None))),
                ]
                for osl, isl in edges:
                    bi = nc.scalar.activation(
                        out=tgt[osl], in_=tgt[isl],
                        func=ACTF.Identity, bias=bias_ap, scale=0.0,
                    )
                    bass._add_dep_helper(
                        bi.ins, sq_inst.ins, sync=True,
                        reason="border fills after the half's ACT stats stream",
                    )

            # --- bias' = bias - (W_s @ mean)/S  (6 accumulated N=1 matmuls) ---
            bps = psum_pool.tile([P, 1], F32, name="bps", tag="ps", bufs=8)
            si = 0
            for p in range(NPAIR):
                nc.tensor.matmul(
                    bps[:],
                    lhsT=wf8[:, hf, p],
                    rhs=mqt[:, hf],
                    start=(si == 0),
                    stop=(si == NSLOT - 1),
                    perf_mode=DR,
                )
                si += 1
            for i in range(NB16):
                nc.tensor.matmul(
                    bps[:],
                    lhsT=wb16[:, hf, i],
                    rhs=mean_bf[:, hf : hf + 1],
                    start=(si == 0),
                    stop=(si == NSLOT - 1),
                )
                si += 1
            nc.vector.scalar_tensor_tensor(
                out=biasp_ch[:, hf : hf + 1],
                in0=bps[:],
                scalar=-1.0 / S,
                in1=bias_ch[:, hf : hf + 1],
                op0=OP.mult,
                op1=OP.add,
            )

            # --- conv: per superblock, 6 slot-instructions x 4 psum tiles ---
            XPITCH = HP * HP  # xqt partition pitch (elements)
            for sb in range(NSB):
                ps = [
                    psum_pool.tile([P, ROWS_PER_MM, W], F32, name="ps", tag="ps", bufs=8)
                    for _ in range(SB_TILES)
                ]
                si = 0
                for p, (t0, t1) in enumerate(FP8_PAIRS):
                    dy0, dx0 = TAPS[t0]
                    dy1, dx1 = TAPS[t1]
                    delta = (dy1 - dy0) * HP + (dx1 - dx0)
                    for k in range(SB_TILES):
                        h0 = sb * SB_ROWS + k * ROWS_PER_MM
                        rhs = bass.AP(
                            tensor=xqt[hf][:].tensor,
                            offset=(h0 + dy0) * HP + dx0,
                            ap=[[XPITCH, P], [delta, 2], [HP, ROWS_PER_MM], [1, W]],
                        )
                        nc.tensor.matmul(
                            ps[k][:],
                            lhsT=wf8[:, hf, p],
                            rhs=rhs,
                            start=(si == 0),
                            stop=(si == NSLOT - 1),
                            perf_mode=DR,
                        )
                    si += 1
                for i, t in enumerate(BF16_TAPS):
                    dy, dx = TAPS[t]
                    for k in range(SB_TILES):
                        h0 = sb * SB_ROWS + k * ROWS_PER_MM
                        nc.tensor.matmul(
                            ps[k][:],
                            lhsT=wb16[:, hf, i],
                            rhs=xbt[hf][:, h0 + dy : h0 + dy + ROWS_PER_MM, dx : dx + W],
                            start=(si == 0),
                            stop=(si == NSLOT - 1),
                        )
                    si += 1
                # epilogue + store in 8-row blocks (2 psum tiles each);
                # alternate ACT/DVE so neither engine bottlenecks
                for half_blk in range(2):
                    stg = stage_pool.tile([P, SB_ROWS // 2, W], BF16, name="stg")
                    for kk in range(2):
                        k = half_blk * 2 + kk
                        dst = stg[:, kk * ROWS_PER_MM : (kk + 1) * ROWS_PER_MM, :]
                        if half_blk == 0:
                            nc.scalar.activation(
                                out=dst,
                                in_=ps[k][:],
                                func=ACTF.Identity,
                                bias=biasp_ch[:, hf : hf + 1],
                                scale=1.0 / S,
                            )
                        else:
                            nc.vector.tensor_scalar(
                                out=dst,
                                in0=ps[k][:],
                                scalar1=1.0 / S,
                                scalar2=biasp_ch[:, hf : hf + 1],
                                op0=OP.mult,
                                op1=OP.add,
                            )
                    out_eng = (nc.gpsimd, nc.sync, nc.scalar)[(sb * 2 + half_blk) % 3]
                    out_eng.dma_start(
                        out=out_ext[
                            hf * P : (hf + 1) * P,
                            sb * SB_ROWS
                            + half_blk * (SB_ROWS // 2) : sb * SB_ROWS
                            + (half_blk + 1) * (SB_ROWS // 2),
                            :,
                        ],
                        in_=stg[:],
                    )

    nc.compile()
    return nc


def get_nc():
    if "nc" not in _CACHED:
        _CACHED["nc"] = build_nc()
    return _CACHED["nc"]


def make_in_maps(x, dw_kernels, pw_kernels, biases):
    x = np.asarray(x, dtype=np.float32)
    dw_kernels = np.asarray(dw_kernels, dtype=np.float32)
    pw_kernels = np.asarray(pw_kernels, dtype=np.float32)
    biases = np.asarray(biases, dtype=np.float32)
    B = x.shape[0]
    in_maps = []
    for i in range(B):
        xq = np.zeros((C, HP, HP), dtype=ml_dtypes.float8_e4m3)
        xb = np.zeros((C, HP, HP), dtype=ml_dtypes.bfloat16)
        xq[:, 1 : 1 + H, 1 : 1 + W] = x[i].astype(ml_dtypes.float8_e4m3)
        xb[:, 1 : 1 + H, 1 : 1 + W] = x[i].astype(ml_dtypes.bfloat16)
        in_maps.append(
            {
                "xq": xq,
                "xb": xb,
                "dw_kernels": np.ascontiguousarray(dw_kernels[i]),
                "pw_kernels": np.ascontiguousarray(pw_kernels[i]),
                "biases": np.ascontiguousarray(biases[i]),
            }
        )
    return in_maps


def postprocess(res, B):
    return np.stack(
        [np.asarray(res.results[i]["out"]).astype(np.float32) for i in range(B)], axis=0
    )


def kernel(x, dw_kernels, pw_kernels, biases):
    B = np.asarray(x).shape[0]
    assert B == 8
    nc = get_nc()
    in_maps = make_in_maps(x, dw_kernels, pw_kernels, biases)
    res = run_bass_kernel_spmd(nc, in_maps, core_ids=list(range(B)))
    return postprocess(res, B)


# revision 26
# speedup vs baseline: 1.4690x; 1.0046x over previous
"""AdaConv2D Trainium2 Bass kernel (fp8-DoubleRow + bf16 hybrid conv).

Problem (per sample): instance-norm(x) -> grouped 3x3 conv (128 groups,
2ch/group, per-sample weights) -> grouped 1x1 conv -> +bias.
B=8, Cin=Cout=256, H=W=128.  Pure data-parallel: 1 sample per NeuronCore.

Math: the 1x1 conv folds into the 3x3 taps (w_eff), the instance norm
folds into the weights (scale per in-channel ci) and bias:
    out = W_s @ x_pad + bias',   W_s[ci,t,co] = w_eff * S/(std_ci+eps)
    bias'[co] = bias[co] - (sum_{ci,t} W_s * mean_ci)/S
with x_pad borders held at mean_ci so border windows cancel, and a
global S=128 pre-scale so fp8-quantized weights stay in e4m3's normal
range (the epilogue multiplies by 1/S).

Precision/speed plan (validated vs f64 reference, ~1.5% L2 global,
gate is 2e-2):
  - Host sends x twice, pre-padded to 130x130: xq = fp8 e4m3 (4.3 MiB)
    and xb = bf16 (8.7 MiB).
  - taps 0..5 run as 3 fp8 DoubleRow matmuls on xq (2 taps per
    instruction; DR costs the same per instruction as one bf16 matmul
    but does 2 taps).
  - taps 6..8 run as bf16 matmuls on xb (near-full precision).
  -> 6 PE instructions per psum tile instead of 9 (bf16-only).
  - Output is written bf16 (8 MiB) and upcast to f32 on the host.

Per-core dataflow:
  - xb streams first (10 chunks/half): DVE accumulates sums, ACT
    accumulates sum-of-squares; the xq stream rides the DMA tail.
  - w_eff scatters (via a zero DRAM scratch) into dense block-diag
    [ci, tap, co] layout, loaded back before stats land; after stats a
    DVE pass scales by S/std and quantizes to the fp8/bf16 lhsT tiles.
  - bias' comes from 6 accumulated N=1 matmuls against the fp8/bf16
    mean, mirroring the conv arithmetic exactly (border cancellation).
  - conv: per 16-row superblock, 4 psum tiles x 6 slot-instructions;
    epilogues alternate ACT/DVE (1/S scale + bias'), emit bf16, DMA out.
"""

import sys

sys.path.insert(0, "/opt/trn_rl_repo")

from contextlib import ExitStack

import numpy as np
import ml_dtypes

from concourse import bacc, bass, mybir, tile
from concourse.bass_utils import run_bass_kernel_spmd

F32 = mybir.dt.float32
BF16 = mybir.dt.bfloat16
FP8 = mybir.dt.float8e4
AX = mybir.AxisListType
OP = mybir.AluOpType
ACTF = mybir.ActivationFunctionType
DR = mybir.MatmulPerfMode.DoubleRow

C = 256          # channels (per sample)
H = W = 128      # spatial
P = 128          # partitions
HP = H + 2       # padded rows/cols (130)
NHF = 2          # channel halves
NCHUNK = 10      # input DMA chunks per half (13 padded rows each)
CHUNK_TR = HP // NCHUNK           # 13 tile rows per chunk
ROWS_PER_MM = 4                   # output rows per psum tile (4*128=512)
SB_TILES = 4                      # psum tiles per superblock
SB_ROWS = ROWS_PER_MM * SB_TILES  # 16 rows per superblock
NSB = H // SB_ROWS                # 8 superblocks per half
NPIX = H * W
EPS = 1e-7
S = 128.0        # weight pre-scale (fp8 range), undone in the epilogue

TAPS = [(t // 3, t % 3) for t in range(9)]
FP8_PAIRS = [(0, 1), (2, 3), (4, 5), (6, 7)]  # DoubleRow tap pairs (xq)
BF16_TAPS = [8]                               # bf16 taps (xb)
NPAIR = len(FP8_PAIRS)
NB16 = len(BF16_TAPS)
NSLOT = NPAIR + NB16

_CACHED = {}


def build_nc():
    nc = bacc.Bacc(trn_type="TRN2")

    xq_ext = nc.declare_dram_parameter("xq", [C, HP, HP], FP8, isOutput=False)
    xb_ext = nc.declare_dram_parameter("xb", [C, HP, HP], BF16, isOutput=False)
    dw_ext = nc.declare_dram_parameter("dw_kernels", [C, 2, 3, 3], F32, isOutput=False)
    pw_ext = nc.declare_dram_parameter("pw_kernels", [C, 2, 1, 1], F32, isOutput=False)
    b_ext = nc.declare_dram_parameter("biases", [C], F32, isOutput=False)
    out_ext = nc.declare_dram_parameter("out", [C, H, W], BF16, isOutput=True)

    # zero-initialized DRAM scratch for the dense block-diag w_eff
    # (runtime scatter only writes the fixed nonzero slots -> idempotent).
    # layout [ci, hf, tap, co] f32
    weff_dram = nc.inline_tensor(
        np.zeros((P, NHF, 9, P), dtype=np.float32), name="weff_zero"
    )
    CI_STRIDE = NHF * 9 * P  # 2304 elements per ci row

    with tile.TileContext(nc) as tc, ExitStack() as ctx:
        const_pool = ctx.enter_context(tc.tile_pool(name="const", bufs=1))
        sq_pool = ctx.enter_context(tc.tile_pool(name="sq", bufs=4))
        psum_pool = ctx.enter_context(tc.tile_pool(name="psum", bufs=8, space="PSUM"))
        stage_pool = ctx.enter_context(tc.tile_pool(name="stage", bufs=6))

        # ---------------- persistent tiles ----------------
        xqt = [const_pool.tile([P, HP, HP], FP8, name=f"xqt{hf}") for hf in range(NHF)]
        xbt = [const_pool.tile([P, HP, HP], BF16, name=f"xbt{hf}") for hf in range(NHF)]

        sums = const_pool.tile([P, NHF, NCHUNK], F32, name="sums")
        sumsqs = const_pool.tile([P, NHF, NCHUNK], F32, name="sumsqs")
        st_a = const_pool.tile([P, NHF], F32, name="st_a")
        st_b = const_pool.tile([P, NHF], F32, name="st_b")
        st_c = const_pool.tile([P, NHF], F32, name="st_c")
        mean_ch = const_pool.tile([P, NHF], F32, name="mean_ch")
        mean_bf = const_pool.tile([P, NHF], BF16, name="mean_bf")
        mqt = const_pool.tile([P, NHF, 2, 1], FP8, name="mqt")
        scS = const_pool.tile([P, NHF], F32, name="scS")
        bias_ch = const_pool.tile([P, NHF], F32, name="bias_ch")
        biasp_ch = const_pool.tile([P, NHF], F32, name="biasp_ch")

        # group-layout weights (partition = group)
        dwg = const_pool.tile([P, 2, 2, 9], F32, name="dwg")    # [g, i, j, t]
        pwg = const_pool.tile([P, 2, 2], F32, name="pwg")       # [g, o, i]
        weffg = const_pool.tile([P, 2, 9, 2], F32, name="weffg")  # [g, j, t, o]

        # dense block-diag weights (per-half tiles so half 0's quantize
        # never picks up a false whole-tile dep on half 1's load)
        weffd = [const_pool.tile([P, 9, P], F32, name=f"weffd{h}") for h in range(NHF)]
        wtmp = [const_pool.tile([P, 9, P], F32, name=f"wtmp{h}") for h in range(NHF)]
        wf8 = const_pool.tile([P, NHF, NPAIR, 2, P], FP8, name="wf8")
        wb16 = const_pool.tile([P, NHF, NB16, P], BF16, name="wb16")

        # ACT LUT warm (sqrt/square/identity) off the critical chains
        zz = const_pool.tile([P, 1], F32, name="zz")
        zz2 = const_pool.tile([P, 1], F32, name="zz2")
        with tc.high_priority():
            nc.vector.memset(zz[:], 0.0)
            nc.scalar.activation(out=zz2[:], in_=zz[:], func=ACTF.Square)
            nc.scalar.sqrt(zz2[:], zz[:])
            nc.scalar.activation(
                out=zz2[:], in_=zz[:], func=ACTF.Identity, bias=zz[:], scale=0.0
            )

        # ------------- early DMAs (no stats dependency) -------------
        # weight-path DMAs live on the gpsimd ring so they never queue
        # behind the x stream (sync) or ACT compute (scalar)
        nc.gpsimd.dma_start(
            out=dwg[:],
            in_=bass.AP(tensor=dw_ext, offset=0, ap=[[36, P], [18, 2], [9, 2], [1, 9]]),
        )
        nc.gpsimd.dma_start(
            out=pwg[:],
            in_=bass.AP(tensor=pw_ext, offset=0, ap=[[4, P], [2, 2], [1, 2]]),
        )

        # ------------- w_eff (group layout) + scatter + load -------------
        with tc.high_priority():
            for o in range(2):
                nc.vector.tensor_scalar(
                    out=weffg[:, :, :, o],
                    in0=dwg[:, 0],
                    scalar1=pwg[:, o, 0:1],
                    scalar2=None,
                    op0=OP.mult,
                )
                nc.vector.scalar_tensor_tensor(
                    out=weffg[:, :, :, o],
                    in0=dwg[:, 1],
                    scalar=pwg[:, o, 1:2],
                    in1=weffg[:, :, :, o],
                    op0=OP.mult,
                    op1=OP.add,
                )

        def emit_scatter_load(hf, scatter_eng):
            # scatter: dst (ci=2a+j, hf, t, co=2a+o) <- weffg[64*hf + a, o, j, t]
            # one 3-dim DMA per j: dst dims (a, t, o), src dims (a, t, o)
            for j in range(2):
                scatter_eng.dma_start(
                    out=bass.AP(
                        tensor=weff_dram,
                        offset=j * CI_STRIDE + hf * 9 * P,
                        ap=[[2 * CI_STRIDE + 2, 64], [P, 9], [1, 2]],
                    ),
                    in_=bass.AP(
                        tensor=weffg[:].tensor,
                        offset=(64 * hf) * 36 + j * 18,
                        ap=[[36, 64], [2, 9], [1, 2]],
                    ),
                )
            # dense load back: weffd[hf][ci, t, co]
            return nc.gpsimd.dma_start(
                out=weffd[hf][:],
                in_=bass.AP(
                    tensor=weff_dram,
                    offset=hf * 9 * P,
                    ap=[[CI_STRIDE, P], [P, 9], [1, P]],
                ),
            )

        # ------------- x input chunks (xb first, xq rides the tail) -------------
        def emit_xb_chunk(hf, ck):
            r0 = ck * CHUNK_TR
            return nc.sync.dma_start(
                out=xbt[hf][:, r0 : r0 + CHUNK_TR, :],
                in_=bass.AP(
                    tensor=xb_ext,
                    offset=hf * P * HP * HP + r0 * HP,
                    ap=[[HP * HP, P], [1, CHUNK_TR * HP]],
                ),
            )

        def emit_xq_chunk(hf, ck):
            r0 = ck * CHUNK_TR
            return nc.sync.dma_start(
                out=xqt[hf][:, r0 : r0 + CHUNK_TR, :],
                in_=bass.AP(
                    tensor=xq_ext,
                    offset=hf * P * HP * HP + r0 * HP,
                    ap=[[HP * HP, P], [1, CHUNK_TR * HP]],
                ),
            )

        with tc.high_priority():
            for ck in range(NCHUNK):
                emit_xb_chunk(0, ck)
            emit_scatter_load(0, nc.gpsimd)
            for ck in range(NCHUNK):
                xq0_last = emit_xq_chunk(0, ck)
            nc.sync.dma_start(
                out=bias_ch[:],
                in_=bass.AP(tensor=b_ext, offset=0, ap=[[1, P], [P, NHF]]),
            )

        # ------------- per-half pipeline -------------
        h0_last_dve = None
        for hf in range(NHF):
            if hf == 1:
                for ck in range(NCHUNK):
                    inst = emit_xb_chunk(1, ck)
                    if ck == 0:
                        bass._add_dep_helper(
                            inst.ins,
                            xq0_last.ins,
                            sync=True,
                            reason="h1 x stream waits for h0 x stream",
                        )
                for ck in range(NCHUNK):
                    emit_xq_chunk(1, ck)
                emit_scatter_load(1, nc.gpsimd)

            # --- per-chunk stats: sums (DVE), sumsq (ACT), both from xb ---
            for ck in range(NCHUNK):
                r0 = max(1, ck * CHUNK_TR)
                r1 = min(1 + H, (ck + 1) * CHUNK_TR)
                gtr = sq_pool.tile([P, CHUNK_TR, W], BF16, name="gtr")
                ts_inst = nc.vector.tensor_scalar(
                    out=gtr[:, 0 : r1 - r0, :],
                    in0=xbt[hf][:, r0:r1, 1 : 1 + W],
                    scalar1=1.0,
                    scalar2=None,
                    op0=OP.mult,
                    op1=OP.add,
                    accum_out=sums[:, hf, ck : ck + 1],
                )
                if hf == 1 and ck == 0 and h0_last_dve is not None:
                    bass._add_dep_helper(
                        ts_inst.ins,
                        h0_last_dve.ins,
                        sync=True,
                        reason="keep h1 DVE stats behind h0 weight quantize",
                    )
                sq = sq_pool.tile([P, CHUNK_TR, W], BF16, name="sq")
                sq_inst = nc.scalar.activation(
                    out=sq[:, 0 : r1 - r0, :],
                    in_=xbt[hf][:, r0:r1, 1 : 1 + W],
                    func=ACTF.Square,
                    accum_out=sumsqs[:, hf, ck : ck + 1],
                )

            # --- stats finalize ---
            with tc.high_priority():
                nc.vector.tensor_reduce(
                    out=st_a[:, hf : hf + 1], in_=sums[:, hf, :], axis=AX.X, op=OP.add
                )
                nc.vector.tensor_scalar(
                    out=mean_ch[:, hf : hf + 1],
                    in0=st_a[:, hf : hf + 1],
                    scalar1=1.0 / NPIX,
                    scalar2=None,
                    op0=OP.mult,
                )
                nc.vector.tensor_reduce(
                    out=st_b[:, hf : hf + 1], in_=sumsqs[:, hf, :], axis=AX.X, op=OP.add
                )
                nc.vector.tensor_tensor(
                    out=st_c[:, hf : hf + 1],
                    in0=mean_ch[:, hf : hf + 1],
                    in1=mean_ch[:, hf : hf + 1],
                    op=OP.mult,
                )
                nc.vector.scalar_tensor_tensor(
                    out=st_b[:, hf : hf + 1],
                    in0=st_c[:, hf : hf + 1],
                    scalar=float(-NPIX),
                    in1=st_b[:, hf : hf + 1],
                    op0=OP.mult,
                    op1=OP.add,
                )
                nc.vector.tensor_scalar(
                    out=st_b[:, hf : hf + 1],
                    in0=st_b[:, hf : hf + 1],
                    scalar1=1.0 / (NPIX - 1),
                    scalar2=None,
                    op0=OP.mult,
                )
                nc.scalar.sqrt(st_b[:, hf : hf + 1], st_b[:, hf : hf + 1])
                # (std + EPS) / S, then reciprocal -> S/(std+EPS)
                nc.vector.tensor_scalar(
                    out=st_b[:, hf : hf + 1],
                    in0=st_b[:, hf : hf + 1],
                    scalar1=EPS,
                    scalar2=1.0 / S,
                    op0=OP.add,
                    op1=OP.mult,
                )
                nc.vector.reciprocal(scS[:, hf : hf + 1], st_b[:, hf : hf + 1])
                nc.vector.tensor_copy(mean_bf[:, hf : hf + 1], mean_ch[:, hf : hf + 1])
                nc.vector.tensor_copy(mqt[:, hf, 0], mean_ch[:, hf : hf + 1])
                nc.vector.tensor_copy(mqt[:, hf, 1], mean_ch[:, hf : hf + 1])

                # --- scale + quantize the dense weights ---
                nc.vector.tensor_scalar(
                    out=wtmp[hf][:],
                    in0=weffd[hf][:],
                    scalar1=scS[:, hf : hf + 1],
                    scalar2=None,
                    op0=OP.mult,
                )
                # fp8 taps -> wf8[hf] ([P, NPAIR*2, P] contiguous)
                nc.vector.tensor_copy(
                    bass.AP(
                        tensor=wf8[:].tensor,
                        offset=hf * NPAIR * 2 * P,
                        ap=[[NHF * NPAIR * 2 * P, P], [P, NPAIR * 2], [1, P]],
                    ),
                    wtmp[hf][:, 0 : 2 * NPAIR, :],
                )
                # bf16 taps
                h0_last_dve = nc.vector.tensor_copy(
                    wb16[:, hf], wtmp[hf][:, 2 * NPAIR : 9, :]
                )

            # --- border fills (bias = mean, scale = 0); pinned after the
            # half's last square so the scheduler can't hoist them into the
            # middle of the ACT stats stream ---
            bias_ap = mean_ch[:, hf : hf + 1]
            for tgt in (xbt[hf], xqt[hf]):
                edges = [
                    ((slice(None), slice(1, 1 + H), 0), (slice(None), slice(1, 1 + H), 1)),
                    ((slice(None), slice(1, 1 + H), HP - 1), (slice(None), slice(1, 1 + H), 1)),
                    ((slice(None), 0, slice(None)), (slice(None), 1, slice(None))),
                    ((slice(None), HP - 1, slice(None)), (slice(None), 1, slice(None))),
                ]
                for osl, isl in edges:
                    bi = nc.scalar.activation(
                        out=tgt[osl], in_=tgt[isl],
                        func=ACTF.Identity, bias=bias_ap, scale=0.0,
                    )
                    bass._add_dep_helper(
                        bi.ins, sq_inst.ins, sync=True,
                        reason="border fills after the half's ACT stats stream",
                    )

            # --- bias' = bias - (W_s @ mean)/S  (6 accumulated N=1 matmuls) ---
            bps = psum_pool.tile([P, 1], F32, name="bps", tag="ps", bufs=8)
            si = 0
            for p in range(NPAIR):
                nc.tensor.matmul(
                    bps[:],
                    lhsT=wf8[:, hf, p],
                    rhs=mqt[:, hf],
                    start=(si == 0),
                    stop=(si == NSLOT - 1),
                    perf_mode=DR,
                )
                si += 1
            for i in range(NB16):
                nc.tensor.matmul(
                    bps[:],
                    lhsT=wb16[:, hf, i],
                    rhs=mean_bf[:, hf : hf + 1],
                    start=(si == 0),
                    stop=(si == NSLOT - 1),
                )
                si += 1
            nc.vector.scalar_tensor_tensor(
                out=biasp_ch[:, hf : hf + 1],
                in0=bps[:],
                scalar=-1.0 / S,
                in1=bias_ch[:, hf : hf + 1],
                op0=OP.mult,
                op1=OP.add,
            )

            # --- conv: per superblock, 6 slot-instructions x 4 psum tiles ---
            XPITCH = HP * HP  # xqt partition pitch (elements)
            for sb in range(NSB):
                ps = [
                    psum_pool.tile([P, ROWS_PER_MM, W], F32, name="ps", tag="ps", bufs=8)
                    for _ in range(SB_TILES)
                ]
                si = 0
                for p, (t0, t1) in enumerate(FP8_PAIRS):
                    dy0, dx0 = TAPS[t0]
                    dy1, dx1 = TAPS[t1]
                    delta = (dy1 - dy0) * HP + (dx1 - dx0)
                    for k in range(SB_TILES):
                        h0 = sb * SB_ROWS + k * ROWS_PER_MM
                        rhs = bass.AP(
                            tensor=xqt[hf][:].tensor,
                            offset=(h0 + dy0) * HP + dx0,
                            ap=[[XPITCH, P], [delta, 2], [HP, ROWS_PER_MM], [1, W]],
                        )
                        nc.tensor.matmul(
                            ps[k][:],
                            lhsT=wf8[:, hf, p],
                            rhs=rhs,
                            start=(si == 0),
                            stop=(si == NSLOT - 1),
                            perf_mode=DR,
                        )
                    si += 1
                for i, t in enumerate(BF16_TAPS):
                    dy, dx = TAPS[t]
                    for k in range(SB_TILES):
                        h0 = sb * SB_ROWS + k * ROWS_PER_MM
                        nc.tensor.matmul(
                            ps[k][:],
                            lhsT=wb16[:, hf, i],
                            rhs=xbt[hf][:, h0 + dy : h0 + dy + ROWS_PER_MM, dx : dx + W],
                            start=(si == 0),
                            stop=(si == NSLOT - 1),
                        )
                    si += 1
                # epilogue + store in 8-row blocks (2 psum tiles each);
                # alternate ACT/DVE so neither engine bottlenecks
                for half_blk in range(2):
                    stg = stage_pool.tile([P, SB_ROWS // 2, W], BF16, name="stg")
                    for kk in range(2):
                        k = half_blk * 2 + kk
                        dst = stg[:, kk * ROWS_PER_MM : (kk + 1) * ROWS_PER_MM, :]
                        if half_blk == 0:
                            nc.scalar.activation(
                                out=dst,
                                in_=ps[k][:],
                                func=ACTF.Identity,
                                bias=biasp_ch[:, hf : hf + 1],
                                scale=1.0 / S,
                            )
                        else:
                            nc.vector.tensor_scalar(
                                out=dst,
                                in0=ps[k][:],
                                scalar1=1.0 / S,
                                scalar2=biasp_ch[:, hf : hf + 1],
                                op0=OP.mult,
                                op1=OP.add,
                            )
                    out_eng = (nc.gpsimd, nc.sync, nc.scalar)[(sb * 2 + half_blk) % 3]
                    out_eng.dma_start(
                        out=out_ext[
                            hf * P : (hf + 1) * P,
                            sb * SB_ROWS
                            + half_blk * (SB_ROWS // 2) : sb * SB_ROWS
                            + (half_blk + 1) * (SB_ROWS // 2),
                            :,
                        ],
                        in_=stg[:],
                    )

    nc.compile()
    return nc


def get_nc():
    if "nc" not in _CACHED:
        _CACHED["nc"] = build_nc()
    return _CACHED["nc"]


def make_in_maps(x, dw_kernels, pw_kernels, biases):
    x = np.asarray(x, dtype=np.float32)
    dw_kernels = np.asarray(dw_kernels, dtype=np.float32)
    pw_kernels = np.asarray(pw_kernels, dtype=np.float32)
    biases = np.asarray(biases, dtype=np.float32)
    B = x.shape[0]
    in_maps = []
    for i in range(B):
        xq = np.zeros((C, HP, HP), dtype=ml_dtypes.float8_e4m3)
        xb = np.zeros((C, HP, HP), dtype=ml_dtypes.bfloat16)
        xq[:, 1 : 1 + H, 1 : 1 + W] = x[i].astype(ml_dtypes.float8_e4m3)
        xb[:, 1 : 1 + H, 1 : 1 + W] = x[i].astype(ml_dtypes.bfloat16)
        in_maps.append(
            {
                "xq": xq,
                "xb": xb,
                "dw_kernels": np.ascontiguousarray(dw_kernels[i]),
                "pw_kernels": np.ascontiguousarray(pw_kernels[i]),
                "biases": np.ascontiguousarray(biases[i]),
            }
        )
    return in_maps


def postprocess(res, B):
    return np.stack(
        [np.asarray(res.results[i]["out"]).astype(np.float32) for i in range(B)], axis=0
    )


def kernel(x, dw_kernels, pw_kernels, biases):
    B = np.asarray(x).shape[0]
    assert B == 8
    nc = get_nc()
    in_maps = make_in_maps(x, dw_kernels, pw_kernels, biases)
    res = run_bass_kernel_spmd(nc, in_maps, core_ids=list(range(B)))
    return postprocess(res, B)


# revision 30
# speedup vs baseline: 1.5224x; 1.0364x over previous
"""AdaConv2D Trainium2 Bass kernel (fp8-DoubleRow + bf16 hybrid conv).

Problem (per sample): instance-norm(x) -> grouped 3x3 conv (128 groups,
2ch/group, per-sample weights) -> grouped 1x1 conv -> +bias.
B=8, Cin=Cout=256, H=W=128.  Pure data-parallel: 1 sample per NeuronCore.

Math: the 1x1 conv folds into the 3x3 taps (w_eff), the instance norm
folds into the weights (scale per in-channel ci) and bias:
    out = W_s @ x_pad + bias',   W_s[ci,t,co] = w_eff * S/(std_ci+eps)
    bias'[co] = bias[co] - (sum_{ci,t} W_s * mean_ci)/S
with x_pad borders held at mean_ci so border windows cancel, and a
global S=128 pre-scale so fp8-quantized weights stay in e4m3's normal
range (the epilogue multiplies by 1/S).

Precision/speed plan (validated vs f64 reference, ~1.5% L2 global,
gate is 2e-2):
  - Host sends x twice, pre-padded to 130x130: xq = fp8 e4m3 (4.3 MiB)
    and xb = bf16 (8.7 MiB).
  - taps 0..5 run as 3 fp8 DoubleRow matmuls on xq (2 taps per
    instruction; DR costs the same per instruction as one bf16 matmul
    but does 2 taps).
  - taps 6..8 run as bf16 matmuls on xb (near-full precision).
  -> 6 PE instructions per psum tile instead of 9 (bf16-only).
  - Output is written bf16 (8 MiB) and upcast to f32 on the host.

Per-core dataflow:
  - xb streams first (10 chunks/half): DVE accumulates sums, ACT
    accumulates sum-of-squares; the xq stream rides the DMA tail.
  - w_eff scatters (via a zero DRAM scratch) into dense block-diag
    [ci, tap, co] layout, loaded back before stats land; after stats a
    DVE pass scales by S/std and quantizes to the fp8/bf16 lhsT tiles.
  - bias' comes from 6 accumulated N=1 matmuls against the fp8/bf16
    mean, mirroring the conv arithmetic exactly (border cancellation).
  - conv: per 16-row superblock, 4 psum tiles x 6 slot-instructions;
    epilogues alternate ACT/DVE (1/S scale + bias'), emit bf16, DMA out.
"""

import sys

sys.path.insert(0, "/opt/trn_rl_repo")

from contextlib import ExitStack

import numpy as np
import ml_dtypes

from concourse import bacc, bass, mybir, tile
from concourse.bass_utils import run_bass_kernel_spmd

F32 = mybir.dt.float32
BF16 = mybir.dt.bfloat16
FP8 = mybir.dt.float8e4
AX = mybir.AxisListType
OP = mybir.AluOpType
ACTF = mybir.ActivationFunctionType
DR = mybir.MatmulPerfMode.DoubleRow

C = 256          # channels (per sample)
H = W = 128      # spatial
P = 128          # partitions
HP = H + 2       # padded rows/cols (130)
NHF = 2          # channel halves
NCHUNK = 10      # input DMA chunks per half (13 padded rows each)
CHUNK_TR = HP // NCHUNK           # 13 tile rows per chunk
ROWS_PER_MM = 4                   # output rows per psum tile (4*128=512)
SB_TILES = 4                      # psum tiles per superblock
SB_ROWS = ROWS_PER_MM * SB_TILES  # 16 rows per superblock
NSB = H // SB_ROWS                # 8 superblocks per half
NPIX = H * W
EPS = 1e-7
S = 128.0        # weight pre-scale (fp8 range), undone in the epilogue

TAPS = [(t // 3, t % 3) for t in range(9)]
FP8_PAIRS = [(0, 1), (2, 3), (4, 5), (6, 7)]  # DoubleRow tap pairs (xq)
BF16_TAPS = [8]                               # bf16 taps (xb)
NPAIR = len(FP8_PAIRS)
NB16 = len(BF16_TAPS)
NSLOT = NPAIR + NB16

_CACHED = {}


def build_nc():
    nc = bacc.Bacc(trn_type="TRN2")

    xq_ext = nc.declare_dram_parameter("xq", [C, HP, HP], FP8, isOutput=False)
    xb_ext = nc.declare_dram_parameter("xb", [C, HP, HP], BF16, isOutput=False)
    dw_ext = nc.declare_dram_parameter("dw_kernels", [C, 2, 3, 3], F32, isOutput=False)
    pw_ext = nc.declare_dram_parameter("pw_kernels", [C, 2, 1, 1], F32, isOutput=False)
    b_ext = nc.declare_dram_parameter("biases", [C], F32, isOutput=False)
    out_ext = nc.declare_dram_parameter("out", [C, H, W], BF16, isOutput=True)

    # zero-initialized DRAM scratch for the dense block-diag w_eff
    # (runtime scatter only writes the fixed nonzero slots -> idempotent).
    # layout [ci, hf, tap, co] f32
    weff_dram = nc.inline_tensor(
        np.zeros((P, NHF, 9, P), dtype=np.float32), name="weff_zero"
    )
    CI_STRIDE = NHF * 9 * P  # 2304 elements per ci row

    with tile.TileContext(nc) as tc, ExitStack() as ctx:
        const_pool = ctx.enter_context(tc.tile_pool(name="const", bufs=1))
        sq_pool = ctx.enter_context(tc.tile_pool(name="sq", bufs=4))
        psum_pool = ctx.enter_context(tc.tile_pool(name="psum", bufs=8, space="PSUM"))
        stage_pool = ctx.enter_context(tc.tile_pool(name="stage", bufs=6))

        # ---------------- persistent tiles ----------------
        xqt = [const_pool.tile([P, HP, HP], FP8, name=f"xqt{hf}") for hf in range(NHF)]
        xbt = [const_pool.tile([P, HP, HP], BF16, name=f"xbt{hf}") for hf in range(NHF)]

        sums = const_pool.tile([P, NHF, NCHUNK], F32, name="sums")
        sumsqs = const_pool.tile([P, NHF, NCHUNK], F32, name="sumsqs")
        st_a = const_pool.tile([P, NHF], F32, name="st_a")
        st_b = const_pool.tile([P, NHF], F32, name="st_b")
        st_c = const_pool.tile([P, NHF], F32, name="st_c")
        mean_ch = const_pool.tile([P, NHF], F32, name="mean_ch")
        mean_bf = const_pool.tile([P, NHF], BF16, name="mean_bf")
        mqt = const_pool.tile([P, NHF, 2, 1], FP8, name="mqt")
        scS = const_pool.tile([P, NHF], F32, name="scS")
        bias_ch = const_pool.tile([P, NHF], F32, name="bias_ch")
        biasp_ch = const_pool.tile([P, NHF], F32, name="biasp_ch")

        # group-layout weights (partition = group)
        dwg = const_pool.tile([P, 2, 2, 9], F32, name="dwg")    # [g, i, j, t]
        pwg = const_pool.tile([P, 2, 2], F32, name="pwg")       # [g, o, i]
        weffg = const_pool.tile([P, 2, 9, 2], F32, name="weffg")  # [g, j, t, o]

        # dense block-diag weights (per-half tiles so half 0's quantize
        # never picks up a false whole-tile dep on half 1's load)
        weffd = [const_pool.tile([P, 9, P], F32, name=f"weffd{h}") for h in range(NHF)]
        wtmp = [const_pool.tile([P, 9, P], F32, name=f"wtmp{h}") for h in range(NHF)]
        wf8 = const_pool.tile([P, NHF, NPAIR, 2, P], FP8, name="wf8")
        wb16 = const_pool.tile([P, NHF, NB16, P], BF16, name="wb16")

        # ACT LUT warm (sqrt/square/identity) off the critical chains
        zz = const_pool.tile([P, 1], F32, name="zz")
        zz2 = const_pool.tile([P, 1], F32, name="zz2")
        with tc.high_priority():
            nc.vector.memset(zz[:], 0.0)
            nc.scalar.activation(out=zz2[:], in_=zz[:], func=ACTF.Square)
            nc.scalar.sqrt(zz2[:], zz[:])
            nc.scalar.activation(
                out=zz2[:], in_=zz[:], func=ACTF.Identity, bias=zz[:], scale=0.0
            )

        # ------------- early DMAs (no stats dependency) -------------
        # weight-path DMAs live on the gpsimd ring so they never queue
        # behind the x stream (sync) or ACT compute (scalar)
        nc.gpsimd.dma_start(
            out=dwg[:],
            in_=bass.AP(tensor=dw_ext, offset=0, ap=[[36, P], [18, 2], [9, 2], [1, 9]]),
        )
        nc.gpsimd.dma_start(
            out=pwg[:],
            in_=bass.AP(tensor=pw_ext, offset=0, ap=[[4, P], [2, 2], [1, 2]]),
        )

        # ------------- w_eff (group layout) + scatter + load -------------
        with tc.high_priority():
            for o in range(2):
                nc.vector.tensor_scalar(
                    out=weffg[:, :, :, o],
                    in0=dwg[:, 0],
                    scalar1=pwg[:, o, 0:1],
                    scalar2=None,
                    op0=OP.mult,
                )
                nc.vector.scalar_tensor_tensor(
                    out=weffg[:, :, :, o],
                    in0=dwg[:, 1],
                    scalar=pwg[:, o, 1:2],
                    in1=weffg[:, :, :, o],
                    op0=OP.mult,
                    op1=OP.add,
                )

        def emit_scatter_load(hf, scatter_eng):
            # scatter: dst (ci=2a+j, hf, t, co=2a+o) <- weffg[64*hf + a, o, j, t]
            # one 3-dim DMA per j: dst dims (a, t, o), src dims (a, t, o)
            for j in range(2):
                scatter_eng.dma_start(
                    out=bass.AP(
                        tensor=weff_dram,
                        offset=j * CI_STRIDE + hf * 9 * P,
                        ap=[[2 * CI_STRIDE + 2, 64], [P, 9], [1, 2]],
                    ),
                    in_=bass.AP(
                        tensor=weffg[:].tensor,
                        offset=(64 * hf) * 36 + j * 18,
                        ap=[[36, 64], [2, 9], [1, 2]],
                    ),
                )
            # dense load back: weffd[hf][ci, t, co]
            return nc.gpsimd.dma_start(
                out=weffd[hf][:],
                in_=bass.AP(
                    tensor=weff_dram,
                    offset=hf * 9 * P,
                    ap=[[CI_STRIDE, P], [P, 9], [1, P]],
                ),
            )

        # ------------- x input chunks (xb first, xq rides the tail) -------------
        def emit_xb_chunk(hf, ck):
            r0 = ck * CHUNK_TR
            return nc.sync.dma_start(
                out=xbt[hf][:, r0 : r0 + CHUNK_TR, :],
                in_=bass.AP(
                    tensor=xb_ext,
                    offset=hf * P * HP * HP + r0 * HP,
                    ap=[[HP * HP, P], [1, CHUNK_TR * HP]],
                ),
            )

        def emit_xq_chunk(hf, ck):
            r0 = ck * CHUNK_TR
            return nc.sync.dma_start(
                out=xqt[hf][:, r0 : r0 + CHUNK_TR, :],
                in_=bass.AP(
                    tensor=xq_ext,
                    offset=hf * P * HP * HP + r0 * HP,
                    ap=[[HP * HP, P], [1, CHUNK_TR * HP]],
                ),
            )

        with tc.high_priority():
            for ck in range(NCHUNK):
                emit_xb_chunk(0, ck)
            emit_scatter_load(0, nc.gpsimd)
            for ck in range(NCHUNK):
                xq0_last = emit_xq_chunk(0, ck)
            nc.sync.dma_start(
                out=bias_ch[:],
                in_=bass.AP(tensor=b_ext, offset=0, ap=[[1, P], [P, NHF]]),
            )

        # ------------- per-half pipeline -------------
        # Emission order = engine queue order.  Both halves' input/stats/
        # finalize are emitted BEFORE half 0's conv loop so that half 1's
        # ACT/DVE stats ops sit ahead of half 0's epilogues in the queues
        # (they are chunk-paced and finish long before the epilogues need
        # the engines); the PE queue still runs h0 conv -> h1 conv.
        h0_last_dve = None
        last_sq = [None, None]

        def emit_input_stats(hf):
            nonlocal h0_last_dve
            if hf == 1:
                for ck in range(NCHUNK):
                    inst = emit_xb_chunk(1, ck)
                    if ck == 0:
                        bass._add_dep_helper(
                            inst.ins,
                            xq0_last.ins,
                            sync=True,
                            reason="h1 x stream waits for h0 x stream",
                        )
                for ck in range(NCHUNK):
                    emit_xq_chunk(1, ck)
                emit_scatter_load(1, nc.gpsimd)

            # --- per-chunk stats: sums (DVE), sumsq (ACT), both from xb ---
            for ck in range(NCHUNK):
                r0 = max(1, ck * CHUNK_TR)
                r1 = min(1 + H, (ck + 1) * CHUNK_TR)
                gtr = sq_pool.tile([P, CHUNK_TR, W], BF16, name="gtr")
                ts_inst = nc.vector.tensor_scalar(
                    out=gtr[:, 0 : r1 - r0, :],
                    in0=xbt[hf][:, r0:r1, 1 : 1 + W],
                    scalar1=1.0,
                    scalar2=None,
                    op0=OP.mult,
                    op1=OP.add,
                    accum_out=sums[:, hf, ck : ck + 1],
                )
                if hf == 1 and ck == 0 and h0_last_dve is not None:
                    bass._add_dep_helper(
                        ts_inst.ins,
                        h0_last_dve.ins,
                        sync=True,
                        reason="keep h1 DVE stats behind h0 weight quantize",
                    )
                sq = sq_pool.tile([P, CHUNK_TR, W], BF16, name="sq")
                last_sq[hf] = nc.scalar.activation(
                    out=sq[:, 0 : r1 - r0, :],
                    in_=xbt[hf][:, r0:r1, 1 : 1 + W],
                    func=ACTF.Square,
                    accum_out=sumsqs[:, hf, ck : ck + 1],
                )

        def emit_finalize(hf):
            nonlocal h0_last_dve
            # --- stats finalize ---
            with tc.high_priority():
                nc.vector.tensor_reduce(
                    out=st_a[:, hf : hf + 1], in_=sums[:, hf, :], axis=AX.X, op=OP.add
                )
                nc.vector.tensor_scalar(
                    out=mean_ch[:, hf : hf + 1],
                    in0=st_a[:, hf : hf + 1],
                    scalar1=1.0 / NPIX,
                    scalar2=None,
                    op0=OP.mult,
                )
                nc.vector.tensor_reduce(
                    out=st_b[:, hf : hf + 1], in_=sumsqs[:, hf, :], axis=AX.X, op=OP.add
                )
                nc.vector.tensor_tensor(
                    out=st_c[:, hf : hf + 1],
                    in0=mean_ch[:, hf : hf + 1],
                    in1=mean_ch[:, hf : hf + 1],
                    op=OP.mult,
                )
                nc.vector.scalar_tensor_tensor(
                    out=st_b[:, hf : hf + 1],
                    in0=st_c[:, hf : hf + 1],
                    scalar=float(-NPIX),
                    in1=st_b[:, hf : hf + 1],
                    op0=OP.mult,
                    op1=OP.add,
                )
                nc.vector.tensor_scalar(
                    out=st_b[:, hf : hf + 1],
                    in0=st_b[:, hf : hf + 1],
                    scalar1=1.0 / (NPIX - 1),
                    scalar2=None,
                    op0=OP.mult,
                )
                nc.scalar.sqrt(st_b[:, hf : hf + 1], st_b[:, hf : hf + 1])
                # (std + EPS) / S, then reciprocal -> S/(std+EPS)
                nc.vector.tensor_scalar(
                    out=st_b[:, hf : hf + 1],
                    in0=st_b[:, hf : hf + 1],
                    scalar1=EPS,
                    scalar2=1.0 / S,
                    op0=OP.add,
                    op1=OP.mult,
                )
                nc.vector.reciprocal(scS[:, hf : hf + 1], st_b[:, hf : hf + 1])
                nc.vector.tensor_copy(mean_bf[:, hf : hf + 1], mean_ch[:, hf : hf + 1])
                nc.vector.tensor_copy(mqt[:, hf, 0], mean_ch[:, hf : hf + 1])
                nc.vector.tensor_copy(mqt[:, hf, 1], mean_ch[:, hf : hf + 1])

                # --- scale + quantize the dense weights ---
                nc.vector.tensor_scalar(
                    out=wtmp[hf][:],
                    in0=weffd[hf][:],
                    scalar1=scS[:, hf : hf + 1],
                    scalar2=None,
                    op0=OP.mult,
                )
                # fp8 taps -> wf8[hf] ([P, NPAIR*2, P] contiguous)
                nc.vector.tensor_copy(
                    bass.AP(
                        tensor=wf8[:].tensor,
                        offset=hf * NPAIR * 2 * P,
                        ap=[[NHF * NPAIR * 2 * P, P], [P, NPAIR * 2], [1, P]],
                    ),
                    wtmp[hf][:, 0 : 2 * NPAIR, :],
                )
                # bf16 taps
                h0_last_dve = nc.vector.tensor_copy(
                    wb16[:, hf], wtmp[hf][:, 2 * NPAIR : 9, :]
                )

            # --- border fills (bias = mean, scale = 0); pinned after the
            # half's last square so the scheduler can't hoist them into the
            # middle of the ACT stats stream ---
            bias_ap = mean_ch[:, hf : hf + 1]
            for tgt in (xbt[hf], xqt[hf]):
                edges = [
                    ((slice(None), slice(1, 1 + H), 0), (slice(None), slice(1, 1 + H), 1)),
                    ((slice(None), slice(1, 1 + H), HP - 1), (slice(None), slice(1, 1 + H), 1)),
                    ((slice(None), 0, slice(None)), (slice(None), 1, slice(None))),
                    ((slice(None), HP - 1, slice(None)), (slice(None), 1, slice(None))),
                ]
                for osl, isl in edges:
                    bi = nc.scalar.activation(
                        out=tgt[osl], in_=tgt[isl],
                        func=ACTF.Identity, bias=bias_ap, scale=0.0,
                    )
                    bass._add_dep_helper(
                        bi.ins, last_sq[hf].ins, sync=True,
                        reason="border fills after the half's ACT stats stream",
                    )

        def emit_conv(hf):
            # --- bias' = bias - (W_s @ mean)/S  (accumulated N=1 matmuls) ---
            bps = psum_pool.tile([P, 1], F32, name="bps", tag="ps", bufs=8)
            si = 0
            for p in range(NPAIR):
                nc.tensor.matmul(
                    bps[:],
                    lhsT=wf8[:, hf, p],
                    rhs=mqt[:, hf],
                    start=(si == 0),
                    stop=(si == NSLOT - 1),
                    perf_mode=DR,
                )
                si += 1
            for i in range(NB16):
                nc.tensor.matmul(
                    bps[:],
                    lhsT=wb16[:, hf, i],
                    rhs=mean_bf[:, hf : hf + 1],
                    start=(si == 0),
                    stop=(si == NSLOT - 1),
                )
                si += 1
            nc.vector.scalar_tensor_tensor(
                out=biasp_ch[:, hf : hf + 1],
                in0=bps[:],
                scalar=-1.0 / S,
                in1=bias_ch[:, hf : hf + 1],
                op0=OP.mult,
                op1=OP.add,
            )

            # --- conv: per superblock, 6 slot-instructions x 4 psum tiles ---
            XPITCH = HP * HP  # xqt partition pitch (elements)
            for sb in range(NSB):
                ps = [
                    psum_pool.tile([P, ROWS_PER_MM, W], F32, name="ps", tag="ps", bufs=8)
                    for _ in range(SB_TILES)
                ]
                si = 0
                for p, (t0, t1) in enumerate(FP8_PAIRS):
                    dy0, dx0 = TAPS[t0]
                    dy1, dx1 = TAPS[t1]
                    delta = (dy1 - dy0) * HP + (dx1 - dx0)
                    for k in range(SB_TILES):
                        h0 = sb * SB_ROWS + k * ROWS_PER_MM
                        rhs = bass.AP(
                            tensor=xqt[hf][:].tensor,
                            offset=(h0 + dy0) * HP + dx0,
                            ap=[[XPITCH, P], [delta, 2], [HP, ROWS_PER_MM], [1, W]],
                        )
                        nc.tensor.matmul(
                            ps[k][:],
                            lhsT=wf8[:, hf, p],
                            rhs=rhs,
                            start=(si == 0),
                            stop=(si == NSLOT - 1),
                            perf_mode=DR,
                        )
                    si += 1
                for i, t in enumerate(BF16_TAPS):
                    dy, dx = TAPS[t]
                    for k in range(SB_TILES):
                        h0 = sb * SB_ROWS + k * ROWS_PER_MM
                        nc.tensor.matmul(
                            ps[k][:],
                            lhsT=wb16[:, hf, i],
                            rhs=xbt[hf][:, h0 + dy : h0 + dy + ROWS_PER_MM, dx : dx + W],
                            start=(si == 0),
                            stop=(si == NSLOT - 1),
                        )
                    si += 1
                # epilogue + store in 8-row blocks (2 psum tiles each);
                # alternate ACT/DVE so neither engine bottlenecks
                for half_blk in range(2):
                    stg = stage_pool.tile([P, SB_ROWS // 2, W], BF16, name="stg")
                    for kk in range(2):
                        k = half_blk * 2 + kk
                        dst = stg[:, kk * ROWS_PER_MM : (kk + 1) * ROWS_PER_MM, :]
                        if half_blk == 0:
                            nc.scalar.activation(
                                out=dst,
                                in_=ps[k][:],
                                func=ACTF.Identity,
                                bias=biasp_ch[:, hf : hf + 1],
                                scale=1.0 / S,
                            )
                        else:
                            nc.vector.tensor_scalar(
                                out=dst,
                                in0=ps[k][:],
                                scalar1=1.0 / S,
                                scalar2=biasp_ch[:, hf : hf + 1],
                                op0=OP.mult,
                                op1=OP.add,
                            )
                    out_eng = (nc.gpsimd, nc.sync)[(sb * 2 + half_blk) % 2]
                    out_eng.dma_start(
                        out=out_ext[
                            hf * P : (hf + 1) * P,
                            sb * SB_ROWS
                            + half_blk * (SB_ROWS // 2) : sb * SB_ROWS
                            + (half_blk + 1) * (SB_ROWS // 2),
                            :,
                        ],
                        in_=stg[:],
                    )

        emit_input_stats(0)
        emit_finalize(0)
        emit_input_stats(1)
        emit_finalize(1)
        emit_conv(0)
        emit_conv(1)

    nc.compile()
    return nc


def get_nc():
    if "nc" not in _CACHED:
        _CACHED["nc"] = build_nc()
    return _CACHED["nc"]


def make_in_maps(x, dw_kernels, pw_kernels, biases):
    x = np.asarray(x, dtype=np.float32)
    dw_kernels = np.asarray(dw_kernels, dtype=np.float32)
    pw_kernels = np.asarray(pw_kernels, dtype=np.float32)
    biases = np.asarray(biases, dtype=np.float32)
    B = x.shape[0]
    in_maps = []
    for i in range(B):
        xq = np.zeros((C, HP, HP), dtype=ml_dtypes.float8_e4m3)
        xb = np.zeros((C, HP, HP), dtype=ml_dtypes.bfloat16)
        xq[:, 1 : 1 + H, 1 : 1 + W] = x[i].astype(ml_dtypes.float8_e4m3)
        xb[:, 1 : 1 + H, 1 : 1 + W] = x[i].astype(ml_dtypes.bfloat16)
        in_maps.append(
            {
                "xq": xq,
                "xb": xb,
                "dw_kernels": np.ascontiguousarray(dw_kernels[i]),
                "pw_kernels": np.ascontiguousarray(pw_kernels[i]),
                "biases": np.ascontiguousarray(biases[i]),
            }
        )
    return in_maps


def postprocess(res, B):
    return np.stack(
        [np.asarray(res.results[i]["out"]).astype(np.float32) for i in range(B)], axis=0
    )


def kernel(x, dw_kernels, pw_kernels, biases):
    B = np.asarray(x).shape[0]
    assert B == 8
    nc = get_nc()
    in_maps = make_in_maps(x, dw_kernels, pw_kernels, biases)
    res = run_bass_kernel_spmd(nc, in_maps, core_ids=list(range(B)))
    return postprocess(res, B)
